# revision 3
# baseline (speedup 1.0000x reference)
"""EvolveGCN (EGCN-H, 2 GRCU layers) Trainium2 Bass kernel, 8-way SPMD.

Strategy (dst-sharded graph parallel, transfer-optimized):
- 8 cores each own a contiguous range of N/8 destination nodes. Edges are
  routed (host-side) to their dst-owner core, grouped by 256-wide dst groups
  and by src half (int16 gather-index limit), padded to a fixed per-group
  subchunk count so the device program is static and identical on all cores.
- Host->device traffic is minimized (the axon tunnel is ~20MB/s):
  nodes are uploaded f32 but SHARDED (N/8 rows per core) and AllGathered
  on-device over NeuronLink; gather indices ship as the 16-row wrapped band
  (replicated to 128 partitions on-device); dst-locations ship fp16;
  edge weights ship int16 fixed-point (w*32768, descaled for free via the
  PSUM->SBUF copy scale); h and the output ship/store fp16.
- Precision split (validated on the exact instance): layer-1 SpMM stays f32
  because the layer-2 top-k selection scores derive from h and the rank-128
  score gap is ~5e-3 - any fp16 rounding upstream of the scores flips the
  selected set and cascades through the sequential matrix GRU (rel err 0.3+).
  Layer-2's data path (h storage, gathers, S_T, both matmuls, output) is
  fp16: selection happens before rounding, everything after is smooth.
  End-to-end sim rel err: 8.8e-4.
- segment_sum linearity: segsum(w * (Z@Q)[src], dst) == segsum(w*Z[src], dst) @ Q.
  Per 128-edge subchunk the core dma_gathers 128 rows of Z, builds the
  weighted one-hot S_T[e, d] = w_e * (dst_e == d) with one fused DVE
  tensor_scalar (is_equal x mult against a constant iota), and accumulates
  G.T = X.T @ S_T in PSUM with one matmul. After a group finishes:
  out = lrelu(G @ Q) via one more matmul.
- Layer boundary: per-step AllGather of the h slices (+ device-computed
  layer-2 scores); on-device exact top-k (vector.max8/max_index + global
  rank by count + indirect rank-scatter) and the 128x128 matrix GRU produce
  layer-2's evolved weights. Layer-1's evolved weights are host-precomputed
  (tiny sequential GRU on pure inputs, replicated - see sharding hint).
"""
import os
import sys

for _p in ("/opt/trn_rl_repo", "/root/.axon_site/_ro/trn_rl_repo"):
    if os.path.isdir(_p) and _p not in sys.path:
        sys.path.insert(0, _p)

import numpy as np

# Persistent jax compilation cache: run_bass_kernel_spmd re-jits a fresh
# closure per call, so without this every invocation re-runs the full
# walrus BIR->NEFF compile (~10-40s). The cache is keyed on the HLO hash;
# the executable still loads + runs on-device per call.
import jax as _jax

for _k, _v in (("jax_compilation_cache_dir", "/tmp/jax_cc_cache"),
               ("jax_persistent_cache_min_compile_time_secs", 0.0),
               ("jax_persistent_cache_min_entry_size_bytes", 0)):
    try:
        _jax.config.update(_k, _v)
    except Exception:
        pass

import concourse.bass as bass
import concourse.bacc as bacc
import concourse.mybir as mybir
import concourse.tile as tile
from concourse.bass_utils import run_bass_kernel_spmd

F32 = mybir.dt.float32
F16 = mybir.dt.float16
I16 = mybir.dt.int16
I32 = mybir.dt.int32
ALU = mybir.AluOpType
ACT = mybir.ActivationFunctionType
SLOPE = float((1.0 / 8.0 + 1.0 / 3.0) / 2.0)  # rrelu eval-mode slope
WSCALE = 32768.0  # edge-weight int16 fixed-point scale


class Cfg:
    def __init__(self, T, N, E, ncores, gw=256, topk_rounds=2):
        self.T, self.N, self.E, self.NCORES = T, N, E, ncores
        assert N % ncores == 0
        self.NPART = N // ncores          # dst nodes per core
        self.GW = gw                      # dst group width (matmul free dim)
        self.NG = -(-self.NPART // gw)    # groups per core
        self.SPLIT = min(32768 - (32768 % 128), -(-N // 2 // 128) * 128)
        if N <= 32767:
            self.SPLIT = -(-N // 2 // 128) * 128  # exercise both halves
        self.SPLIT = min(self.SPLIT, N)
        self.D = 128
        self.K = 128
        self.C_SC = -(-N // 128)          # score columns per partition
        self.PADN = 128 * self.C_SC
        self.R = topk_rounds              # rounds of per-partition max8
        self.NCAND = 128 * 8 * topk_rounds
        self.F_GH = None                  # set from data
        self.ncol = None
        self.ncol8 = None

    def set_fgh(self, f):
        self.F_GH = f
        self.ncol = self.NG * 2 * f           # metadata columns per t
        self.ncol8 = self.NG * 2 * f * 8      # idx columns per t


# ---------------------------------------------------------------- host prep
def _pack_edges(cfg, edge_src, edge_dst, edge_w):
    """Per-core static streams. Returns (idx, dstloc, w) arrays:
    idx   [NCORES, T, 16, ncol8]  int16   (16-row wrapped band)
    dstloc[NCORES, T, 128, ncol]  float16 (values 0..GW-1, exact in fp16)
    wpack [NCORES, T, 128, ncol]  int16   (w * 32768 fixed-point)
    """
    T, NG, GW, NPART, SPLIT = cfg.T, cfg.NG, cfg.GW, cfg.NPART, cfg.SPLIT
    NC = cfg.NCORES
    # fixed subchunks per (group, half): global max over all (core,t,g,half)
    maxc = 0
    percore_key = []
    for t in range(T):
        dst, src = edge_dst[t], edge_src[t]
        key = ((dst // NPART) * NG + (dst % NPART) // GW) * 2 + (src >= SPLIT)
        percore_key.append(key.astype(np.int64))
        maxc = max(maxc, int(np.bincount(key, minlength=NC * NG * 2).max()))
    F = -(-maxc // 128)
    cfg.set_fgh(F)

    idx = np.zeros((NC, T, 16, cfg.ncol8), np.int16)
    dstloc = np.zeros((NC, T, 128, cfg.ncol), np.float16)
    wpack = np.zeros((NC, T, 128, cfg.ncol), np.int16)
    for t in range(T):
        dst, src, w = edge_dst[t], edge_src[t], edge_w[t]
        key = percore_key[t]
        order = np.argsort(key, kind="stable")
        key_s = key[order]
        src_s, dst_s, w_s = src[order], dst[order], w[order]
        cnt = np.bincount(key_s, minlength=NC * NG * 2)
        start = np.concatenate([[0], np.cumsum(cnt)[:-1]])
        pos = np.arange(len(key_s)) - start[key_s]  # position within block
        core = key_s // (NG * 2)
        blk = key_s % (NG * 2)                      # (g*2+half) within core
        i = pos                                     # stream slot in block
        s_sub, p_row = i // 128, i % 128
        col = blk * F + s_sub
        dl = (dst_s % NPART) % GW
        half = blk % 2
        iv = src_s - half * SPLIT
        assert iv.max() < 32768 and iv.min() >= 0
        dstloc[core, t, p_row, col] = dl.astype(np.float16)
        wpack[core, t, p_row, col] = np.minimum(
            np.round(w_s.astype(np.float64) * WSCALE), WSCALE - 1).astype(np.int16)
        # 16-row wrapped indices
        r = i % 16
        j = blk * F * 8 + i // 16
        idx[core, t, r, j] = iv.astype(np.int16)
    return idx, dstloc, wpack


def _host_gru_layer1(cfg, nodes, W_init, scorer, gW, gU, gb):
    """Exact fp32 replica of the reference layer-1 weight evolution."""
    sn = np.float32(np.linalg.norm(scorer))
    Q = W_init.copy()
    qns = []
    for t in range(cfg.T):
        Z = nodes[t]
        scores = (Z @ scorer)[:, 0] / sn
        idx = np.argsort(-scores, kind="stable")[: cfg.K]
        z_topk = (Z[idx] * np.tanh(scores[idx])[:, None]).T
        upd = 1.0 / (1.0 + np.exp(-(gW[0] @ z_topk + gU[0] @ Q + gb[0])))
        rst = 1.0 / (1.0 + np.exp(-(gW[1] @ z_topk + gU[1] @ Q + gb[1])))
        h_cap = np.tanh(gW[2] @ z_topk + gU[2] @ (rst * Q) + gb[2])
        Q = (1.0 - upd) * Q + upd * h_cap
        qns.append(Q.copy())
    return np.stack(qns).astype(np.float32)


# ---------------------------------------------------------------- device build
def _build(cfg):
    nc = bacc.Bacc("TRN2", target_bir_lowering=False, debug=False,
                   num_devices=cfg.NCORES)
    T, N, D, GW, NG, F, NPART = cfg.T, cfg.N, cfg.D, cfg.GW, cfg.NG, cfg.F_GH, cfg.NPART
    SPLIT, C_SC, PADN, R = cfg.SPLIT, cfg.C_SC, cfg.PADN, cfg.R
    NCAND = cfg.NCAND
    core_ids = list(range(cfg.NCORES))

    def dram_in(name, shape, dtype=F32):
        return nc.dram_tensor(name, list(shape), dtype, kind="ExternalInput").ap()

    nodes_sh = dram_in("nodes_sh", (T, NPART, D))
    qn1 = dram_in("qn1", (T, D, D))
    gWT2 = dram_in("gWT2", (3, D, D))
    gUT2 = dram_in("gUT2", (3, D, D))
    gb2 = dram_in("gb2", (3, D, D))
    winit2 = dram_in("winit2", (D, D))
    scorer2 = dram_in("scorer2", (D, 1))          # pre-normalized
    iota_gw = dram_in("iota_gw", (128, GW))       # row = 0..GW-1, all partitions
    iota_col = dram_in("iota_col", (128, 1))      # p * C_SC
    identity = dram_in("identity", (128, 128))
    negpad = dram_in("negpad", (1, 128))          # -1e30 row
    idx_d = dram_in("idx", (T, 16, cfg.ncol8), I16)
    dstloc_d = dram_in("dstloc", (T, 128, cfg.ncol), F16)
    w_d = dram_in("wv", (T, 128, cfg.ncol), I16)
    out_d = nc.dram_tensor("out", [T, NPART, D], F16, kind="ExternalOutput").ap()

    # group geometry: (g, list of (row0, width<=128)) covering [g*GW, min(..,NPART))
    geom = []
    for g in range(NG):
        r0 = g * GW
        r1 = min(r0 + GW, NPART)
        hh = []
        x = r0
        while x < r1:
            wdt = min(128, r1 - x)
            hh.append((x, wdt))
            x += wdt
        geom.append((r0, hh))

    with tile.TileContext(nc) as tc:
        import contextlib
        ctx = contextlib.ExitStack()
        with ctx:
            sb = ctx.enter_context(tc.tile_pool(name="sb", bufs=1))
            meta = ctx.enter_context(tc.tile_pool(name="meta", bufs=1))
            xgp = ctx.enter_context(tc.tile_pool(name="xgp", bufs=4))
            stp = ctx.enter_context(tc.tile_pool(name="stp", bufs=8))
            gtp = ctx.enter_context(tc.tile_pool(name="gtp", bufs=3))
            drp = ctx.enter_context(tc.tile_pool(name="drp", bufs=4))
            psg = ctx.enter_context(tc.tile_pool(name="psg", bufs=2, space="PSUM"))
            pso = ctx.enter_context(tc.tile_pool(name="pso", bufs=1, space="PSUM"))
            tkp = ctx.enter_context(tc.tile_pool(name="tkp", bufs=1))
            dram = ctx.enter_context(tc.tile_pool(name="dram", bufs=1, space="DRAM"))

            # constants
            iota_sb = sb.tile([128, GW], F32, tag="iota")
            nc.sync.dma_start(out=iota_sb[:], in_=iota_gw[:])
            ident_sb = sb.tile([128, 128], F32, tag="ident")
            nc.sync.dma_start(out=ident_sb[:], in_=identity[:])
            iotac_sb = sb.tile([128, 1], F32, tag="iotac")
            nc.sync.dma_start(out=iotac_sb[:], in_=iota_col[:])
            neg_sb = sb.tile([1, 128], F32, tag="negp")
            nc.sync.dma_start(out=neg_sb[:], in_=negpad[:])
            sc2_sb = sb.tile([128, 1], F32, tag="sc2")
            nc.sync.dma_start(out=sc2_sb[:], in_=scorer2[:])
            gW_sb, gU_sb, gb_sb = [], [], []
            for i in range(3):
                a = sb.tile([128, 128], F32, name=f"gw{i}", tag=f"gw{i}")
                nc.sync.dma_start(out=a[:], in_=gWT2[i])
                gW_sb.append(a)
                b = sb.tile([128, 128], F32, name=f"gu{i}", tag=f"gu{i}")
                nc.sync.dma_start(out=b[:], in_=gUT2[i])
                gU_sb.append(b)
                c = sb.tile([128, 128], F32, name=f"gb{i}", tag=f"gb{i}")
                nc.sync.dma_start(out=c[:], in_=gb2[i])
                gb_sb.append(c)
            qn1_sb = []
            for t in range(T):
                q = sb.tile([128, 128], F32, name=f"qn1_{t}", tag=f"qn1_{t}")
                nc.sync.dma_start(out=q[:], in_=qn1[t])
                qn1_sb.append(q)

            # persistent DRAM buffers
            nodes_sl = [dram.tile([NPART, D], F32, name=f"nsl{t}", tag=f"nsl{t}")
                        for t in range(T)]
            nodes_full = [dram.tile([N, D], F32, name=f"nfl{t}", tag=f"nfl{t}",
                                    addr_space="Shared") for t in range(T)]
            nodes_loc = [dram.tile([N, D], F32, name=f"nlc{t}", tag=f"nlc{t}")
                         for t in range(T)]
            h_slice = [dram.tile([NPART, D], F16, name=f"hsl{t}", tag=f"hsl{t}")
                       for t in range(T)]
            h_full = [dram.tile([N, D], F16, name=f"hfl{t}", tag=f"hfl{t}",
                                addr_space="Shared") for t in range(T)]
            sc_slice = [dram.tile([1, NPART], F32, name=f"ssl{t}", tag=f"ssl{t}")
                        for t in range(T)]
            h_loc = [dram.tile([N, D], F16, name=f"hlc{t}", tag=f"hlc{t}")
                     for t in range(T)]
            sc_full = [dram.tile([1, PADN], F32, name=f"sfl{t}", tag=f"sfl{t}",
                                 addr_space="Shared") for t in range(T)]
            cand_dram = dram.tile([1, NCAND], F32, tag="cand", bufs=2)
            sorted_dram = dram.tile([129, 2], F32, tag="sorted", bufs=2)

            qn2_sb = [sb.tile([128, 128], F32, name=f"qn2_{t}", tag=f"qn2_{t}")
                      for t in range(T)]
            qn2h_sb = [sb.tile([128, 128], F16, name=f"qn2h_{t}", tag=f"qn2h_{t}")
                       for t in range(T)]

            # reassemble full nodes on-device: shard -> (bounce) -> AllGather
            for t in range(T):
                nc.sync.dma_start(out=nodes_sl[t][:], in_=nodes_sh[t])
                nc.gpsimd.collective_compute(
                    "AllGather", ALU.bypass,
                    replica_groups=[core_ids],
                    ins=[nodes_sl[t][:].opt()],
                    outs=[nodes_full[t][:].opt()])
                nc.sync.dma_start(out=nodes_loc[t][:], in_=nodes_full[t][:])

            def spmm_pass(t, z_src_ap, qn_tile, layer):
                """One (layer, t) SpMM pass. z_src_ap: [N, D] DRAM AP
                (f32 for layer 1, fp16 for layer 2)."""
                zdt = F32 if layer == 1 else F16
                idx_sb = meta.tile([128, cfg.ncol8], I16, tag="idx")
                for s in range(8):
                    nc.sync.dma_start(out=idx_sb[16 * s:16 * (s + 1), :],
                                      in_=idx_d[t])
                dl16_sb = meta.tile([128, cfg.ncol], F16, tag="dl16")
                nc.sync.dma_start(out=dl16_sb[:], in_=dstloc_d[t])
                dl_sb = meta.tile([128, cfg.ncol], F32, tag="dl")
                nc.vector.tensor_copy(out=dl_sb[:], in_=dl16_sb[:])
                wq_sb = meta.tile([128, cfg.ncol], I16, tag="wq")
                nc.sync.dma_start(out=wq_sb[:], in_=w_d[t])
                w_sb = meta.tile([128, cfg.ncol], F32, tag="wv")
                nc.vector.tensor_copy(out=w_sb[:], in_=wq_sb[:])
                z_lo = z_src_ap[0:SPLIT, :]
                z_hi = z_src_ap[SPLIT:N, :]
                for g in range(NG):
                    r0, hh = geom[g]
                    xg = []
                    for half, zsrc in ((0, z_lo), (1, z_hi)):
                        xt = xgp.tile([128, F * 128], zdt, tag="xg",
                                      name=f"xg{layer}_{t}_{g}_{half}")
                        c0 = (g * 2 + half) * F * 8
                        # single_packet SWDGE limit: <=64 desc/engine -> 1024 idxs
                        for s0 in range(0, F, 8):
                            ns = min(8, F - s0)
                            nc.gpsimd.dma_gather(
                                out_ap=xt[:, s0 * 128:(s0 + ns) * 128]
                                .rearrange("p (s e) -> p s e", e=128),
                                in_ap=zsrc,
                                idxs_ap=idx_sb[:, c0 + s0 * 8:c0 + (s0 + ns) * 8],
                                num_idxs=ns * 128,
                                num_idxs_reg=ns * 128,
                                elem_size=128,
                            )
                        xg.append(xt)
                    gt_ps = psg.tile([128, GW], F32, tag="gt", space="PSUM")
                    nmm = 2 * F
                    k = 0
                    for half in (0, 1):
                        for s in range(F):
                            col = (g * 2 + half) * F + s
                            st = stp.tile([128, GW], zdt, tag="st",
                                          name=f"st{layer}_{t}_{g}_{half}_{s}")
                            nc.vector.tensor_scalar(
                                out=st[:], in0=iota_sb[:],
                                scalar1=dl_sb[:, col:col + 1],
                                scalar2=w_sb[:, col:col + 1],
                                op0=ALU.is_equal, op1=ALU.mult)
                            lhs = xg[half][:, s * 128:(s + 1) * 128]
                            nc.tensor.matmul(out=gt_ps[:], lhsT=lhs, rhs=st[:],
                                             start=(k == 0), stop=(k == nmm - 1))
                            k += 1
                    # copy-out descales the int16 fixed-point edge weights
                    gt_sb = gtp.tile([128, GW], zdt, tag="gts")
                    nc.scalar.activation(out=gt_sb[:], in_=gt_ps[:], func=ACT.Copy,
                                         scale=float(1.0 / WSCALE))
                    for (rr, wdt) in hh:
                        o_ps = pso.tile([128, 128], F32, tag="ops", space="PSUM", bufs=2)
                        lhs2 = gt_sb[:, rr - r0:rr - r0 + wdt]
                        rhs2 = qn_tile[:]
                        nc.tensor.matmul(out=o_ps[:wdt, :], lhsT=lhs2, rhs=rhs2,
                                         start=True, stop=True)
                        sx = drp.tile([128, 128], F32, tag="sx")
                        nc.scalar.activation(out=sx[:wdt, :], in_=o_ps[:wdt, :],
                                             func=ACT.Copy, scale=SLOPE)
                        hb = drp.tile([128, 128], F32, tag="hb")
                        nc.vector.tensor_tensor(out=hb[:wdt, :], in0=o_ps[:wdt, :],
                                                in1=sx[:wdt, :], op=ALU.max)
                        hb16 = drp.tile([128, 128], F16, tag="hb16")
                        nc.vector.tensor_copy(out=hb16[:wdt, :], in_=hb[:wdt, :])
                        if layer == 1:
                            nc.sync.dma_start(out=h_slice[t][rr:rr + wdt, :],
                                              in_=hb16[:wdt, :])
                            # scores2 slice: transpose h then scorer2 matvec
                            ht_ps = pso.tile([128, 128], F32, tag="htp",
                                             space="PSUM")
                            nc.tensor.transpose(out=ht_ps[:, :wdt], in_=hb[:wdt, :],
                                                identity=ident_sb[:wdt, :wdt])
                            ht_sb = drp.tile([128, 128], F32, tag="hts")
                            nc.scalar.activation(out=ht_sb[:, :wdt],
                                                 in_=ht_ps[:, :wdt], func=ACT.Copy)
                            s_ps = pso.tile([1, 128], F32, tag="sps", space="PSUM")
                            nc.tensor.matmul(out=s_ps[:, :wdt], lhsT=sc2_sb[:],
                                             rhs=ht_sb[:, :wdt], start=True,
                                             stop=True)
                            s_sb = drp.tile([1, 128], F32, tag="ssb")
                            nc.scalar.activation(out=s_sb[:, :wdt],
                                                 in_=s_ps[:, :wdt], func=ACT.Copy)
                            nc.sync.dma_start(
                                out=sc_slice[t][:, rr:rr + wdt],
                                in_=s_sb[:1, :wdt])
                        else:
                            nc.sync.dma_start(out=out_d[t, rr:rr + wdt, :],
                                              in_=hb16[:wdt, :])
                if layer == 1:
                    nc.gpsimd.collective_compute(
                        "AllGather", ALU.bypass,
                        replica_groups=[core_ids],
                        ins=[h_slice[t][:].opt()],
                        outs=[h_full[t][:].opt()])
                    nc.sync.dma_start(out=h_loc[t][:], in_=h_full[t][:])
                    nc.gpsimd.collective_compute(
                        "AllGather", ALU.bypass,
                        replica_groups=[core_ids],
                        ins=[sc_slice[t][:].opt()],
                        outs=[sc_full[t][:, 0:N].opt()])

            def topk_gru(t, q_prev):
                """Exact top-128 of sc_full[t] + matrix GRU -> qn2_sb[t]."""
                S = tkp.tile([128, C_SC], F32, tag="S")
                nc.sync.dma_start(out=S[:],
                                  in_=sc_full[t][:].rearrange("o (p c) -> (o p) c",
                                                              c=C_SC))
                if PADN > N:
                    p_t, c_t = N // C_SC, N % C_SC
                    nc.sync.dma_start(out=S[p_t:p_t + 1, c_t:C_SC],
                                      in_=negpad[0:1, 0:C_SC - c_t])
                    if p_t + 1 < 128:
                        nc.sync.dma_start(
                            out=S[p_t + 1:128, :],
                            in_=negpad[0:1, 0:1].to_broadcast(
                                [127 - p_t, C_SC]))
                vals = tkp.tile([128, 8 * R], F32, tag="vals")
                cols = tkp.tile([128, 8 * R], F32, tag="cols")
                Swork = S
                for r in range(R):
                    mx = tkp.tile([128, 8], F32, tag="mx")
                    nc.vector.max(out=mx[:], in_=Swork[:])
                    ix = tkp.tile([128, 8], mybir.dt.uint32, tag="ix")
                    nc.vector.max_index(out=ix[:], in_max=mx[:], in_values=Swork[:])
                    nc.vector.tensor_copy(out=vals[:, r * 8:(r + 1) * 8], in_=mx[:])
                    nc.vector.tensor_copy(out=cols[:, r * 8:(r + 1) * 8], in_=ix[:])
                    if r < R - 1:
                        S2 = tkp.tile([128, C_SC], F32, tag=f"Sw{r % 2}")
                        nc.vector.match_replace(out=S2[:], in_to_replace=mx[:],
                                                in_values=Swork[:],
                                                imm_value=-1e30)
                        Swork = S2
                # global node id n = p*C_SC + col
                nid = tkp.tile([128, 8 * R], F32, tag="nid")
                nc.vector.tensor_scalar(out=nid[:], in0=cols[:],
                                        scalar1=iotac_sb[:, :1], scalar2=None,
                                        op0=ALU.add)
                # broadcast all candidates to all partitions via DRAM bounce
                nc.sync.dma_start(out=cand_dram[:], in_=vals[:])
                cb = tkp.tile([128, NCAND], F32, tag="cb")
                nc.sync.dma_start(out=cb[:],
                                  in_=cand_dram[:].to_broadcast([128, NCAND]))
                rank = tkp.tile([128, 8 * R], F32, tag="rank")
                for j in range(8 * R):
                    cmp = tkp.tile([128, NCAND], F32, tag="cmp")
                    nc.vector.tensor_scalar(out=cmp[:], in0=cb[:],
                                            scalar1=vals[:, j:j + 1], scalar2=None,
                                            op0=ALU.is_gt)
                    nc.vector.tensor_reduce(out=rank[:, j:j + 1], in_=cmp[:],
                                            axis=mybir.AxisListType.X, op=ALU.add)
                nc.vector.tensor_scalar(out=rank[:], in0=rank[:], scalar1=128.0,
                                        scalar2=None, op0=ALU.min)
                ri = tkp.tile([128, 8 * R], I32, tag="ri")
                nc.vector.tensor_copy(out=ri[:], in_=rank[:])
                pairs = tkp.tile([128, 16 * R], F32, tag="pairs")
                nc.vector.tensor_copy(
                    out=pairs[:].rearrange("p (j two) -> p j two", two=2)[:, :, 0],
                    in_=nid[:])
                nc.vector.tensor_copy(
                    out=pairs[:].rearrange("p (j two) -> p j two", two=2)[:, :, 1],
                    in_=vals[:])
                for j in range(8 * R):
                    nc.gpsimd.indirect_dma_start(
                        out=sorted_dram[:],
                        out_offset=bass.IndirectOffsetOnAxis(
                            ap=ri[:, j:j + 1], axis=0),
                        in_=pairs[:, 2 * j:2 * j + 2],
                        in_offset=None)
                sv = tkp.tile([128, 2], F32, tag="sv")
                nc.sync.dma_start(out=sv[:], in_=sorted_dram[0:128, :])
                nidx = tkp.tile([128, 1], I32, tag="nidx")
                nc.vector.tensor_copy(out=nidx[:], in_=sv[:, 0:1])
                tanhv = tkp.tile([128, 1], F32, tag="tanhv")
                nc.scalar.activation(out=tanhv[:], in_=sv[:, 1:2], func=ACT.Tanh)
                zsel16 = tkp.tile([128, 128], F16, tag="zsel16")
                nc.gpsimd.indirect_dma_start(
                    out=zsel16[:], out_offset=None,
                    in_=h_full[t][:],
                    in_offset=bass.IndirectOffsetOnAxis(ap=nidx[:, :1], axis=0))
                zsel = tkp.tile([128, 128], F32, tag="zsel")
                nc.vector.tensor_copy(out=zsel[:], in_=zsel16[:])
                zs2 = tkp.tile([128, 128], F32, tag="zs2")
                nc.scalar.activation(out=zs2[:], in_=zsel[:], func=ACT.Copy,
                                     scale=tanhv[:, :1])
                zt_ps = pso.tile([128, 128], F32, tag="ztp", space="PSUM")
                nc.tensor.transpose(out=zt_ps[:], in_=zs2[:], identity=ident_sb[:])
                ztop = tkp.tile([128, 128], F32, tag="ztop")
                nc.scalar.activation(out=ztop[:], in_=zt_ps[:], func=ACT.Copy)
                # matrix GRU
                gates = []
                rstq = None
                for i in range(3):
                    g_ps = pso.tile([128, 128], F32, tag="gps", space="PSUM")
                    nc.tensor.matmul(out=g_ps[:], lhsT=gW_sb[i][:], rhs=ztop[:],
                                     start=True, stop=False)
                    other = q_prev if i < 2 else rstq
                    nc.tensor.matmul(out=g_ps[:], lhsT=gU_sb[i][:], rhs=other[:],
                                     start=False, stop=True)
                    gsum = tkp.tile([128, 128], F32, tag=f"gsum{i}")
                    nc.vector.tensor_tensor(out=gsum[:], in0=g_ps[:],
                                            in1=gb_sb[i][:], op=ALU.add)
                    gact = tkp.tile([128, 128], F32, tag=f"gact{i}")
                    nc.scalar.activation(out=gact[:], in_=gsum[:],
                                         func=(ACT.Sigmoid if i < 2 else ACT.Tanh))
                    gates.append(gact)
                    if i == 1:
                        rstq = tkp.tile([128, 128], F32, tag="rstq")
                        nc.vector.tensor_tensor(out=rstq[:], in0=gates[1][:],
                                                in1=q_prev[:], op=ALU.mult)
                upd, h_cap = gates[0], gates[2]
                dql = tkp.tile([128, 128], F32, tag="dql")
                nc.vector.tensor_tensor(out=dql[:], in0=h_cap[:], in1=q_prev[:],
                                        op=ALU.subtract)
                udl = tkp.tile([128, 128], F32, tag="udl")
                nc.vector.tensor_tensor(out=udl[:], in0=upd[:], in1=dql[:],
                                        op=ALU.mult)
                nc.vector.tensor_tensor(out=qn2_sb[t][:], in0=q_prev[:],
                                        in1=udl[:], op=ALU.add)
                nc.vector.tensor_copy(out=qn2h_sb[t][:], in_=qn2_sb[t][:])
                return qn2_sb[t]

            # ---- program ----
            bisect = os.environ.get("KBISECT", "")
            if bisect.startswith("spmm"):
                npass = int(bisect[4:] or 2 * T)
                for i in range(npass):
                    spmm_pass(i % T, nodes_loc[i % T][:], qn1_sb[i % T], layer=1)
            else:
                for t in range(T):
                    spmm_pass(t, nodes_loc[t][:], qn1_sb[t], layer=1)
                qprev = sb.tile([128, 128], F32, name="winit2_sb", tag="winit2")
                nc.sync.dma_start(out=qprev[:], in_=winit2[:])
                for t in range(T):
                    qprev = topk_gru(t, qprev)
                for t in range(T):
                    spmm_pass(t, h_loc[t][:], qn2h_sb[t], layer=2)

    nc.compile()
    return nc


# ---------------------------------------------------------------- entry point
_CACHE = {}
_LAST_IN_MAPS = None

# full-problem constants (hardcoded per contract)
_T, _N, _E, _NCORES = 6, 50000, 1600000, 8


def kernel(nodes, edge_src, edge_dst, edge_weight,
           W_init1, scorer1, gate_W1, gate_U1, gate_b1,
           W_init2, scorer2, gate_W2, gate_U2, gate_b2):
    nodes = np.ascontiguousarray(np.asarray(nodes, np.float32))
    T, N, D = nodes.shape
    E = np.asarray(edge_src).shape[1]
    gw = int(os.environ.get("KGW", "256"))
    cfg = Cfg(T, N, E, _NCORES, gw=gw, topk_rounds=2)
    idx, dstloc, wpack = _pack_edges(
        cfg, np.asarray(edge_src), np.asarray(edge_dst),
        np.asarray(edge_weight, np.float32))
    qn1 = _host_gru_layer1(cfg, nodes, np.asarray(W_init1, np.float32),
                           np.asarray(scorer1, np.float32),
                           np.asarray(gate_W1, np.float32),
                           np.asarray(gate_U1, np.float32),
                           np.asarray(gate_b1, np.float32))
    key = (T, N, E, cfg.F_GH, cfg.GW, cfg.R)
    if key not in _CACHE:
        _CACHE[key] = _build(cfg)
    nc = _CACHE[key]

    sc2n = (np.asarray(scorer2, np.float32)
            / np.float32(np.linalg.norm(scorer2))).astype(np.float32)
    shared = {
        "qn1": qn1,
        "gWT2": np.ascontiguousarray(
            np.transpose(np.asarray(gate_W2, np.float32), (0, 2, 1))),
        "gUT2": np.ascontiguousarray(
            np.transpose(np.asarray(gate_U2, np.float32), (0, 2, 1))),
        "gb2": np.asarray(gate_b2, np.float32),
        "winit2": np.asarray(W_init2, np.float32),
        "scorer2": sc2n,
        "iota_gw": np.tile(np.arange(cfg.GW, dtype=np.float32), (128, 1)),
        "iota_col": (np.arange(128, dtype=np.float32) * cfg.C_SC)[:, None],
        "identity": np.eye(128, dtype=np.float32),
        "negpad": np.full((1, 128), -1e30, np.float32),
    }
    in_maps = []
    for c in range(cfg.NCORES):
        m = dict(shared)
        m["nodes_sh"] = np.ascontiguousarray(
            nodes[:, c * cfg.NPART:(c + 1) * cfg.NPART, :])
        m["idx"] = idx[c]
        m["dstloc"] = dstloc[c]
        m["wv"] = wpack[c]
        in_maps.append(m)
    global _LAST_IN_MAPS
    _LAST_IN_MAPS = in_maps
    res = run_bass_kernel_spmd(nc, in_maps, list(range(cfg.NCORES)))
    out = np.concatenate([res.results[c]["out"] for c in range(cfg.NCORES)],
                         axis=1)
    return out.astype(np.float32)


# revision 18
# speedup vs baseline: 2.7038x; 2.7038x over previous
"""EvolveGCN (EGCN-H, 2 GRCU layers) Trainium2 Bass kernel, 8-way SPMD.

Strategy (dst-sharded graph parallel, transfer-optimized):
- 8 cores each own a contiguous range of N/8 destination nodes. Edges are
  routed (host-side) to their dst-owner core, grouped by 256-wide dst groups
  and by src half (int16 gather-index limit), padded to a fixed per-group
  subchunk count so the device program is static and identical on all cores.
- Host->device traffic is minimized (the axon tunnel is ~20MB/s):
  nodes are uploaded f32 but SHARDED (N/8 rows per core) and AllGathered
  on-device over NeuronLink; gather indices ship as the 16-row wrapped band
  (replicated to 128 partitions on-device); dst-locations ship fp16;
  edge weights ship int16 fixed-point (w*32768, descaled for free via the
  PSUM->SBUF copy scale); h and the output ship/store fp16.
- Precision split (validated on the exact instance): layer-1 SpMM stays f32
  because the layer-2 top-k selection scores derive from h and the rank-128
  score gap is ~5e-3 - any fp16 rounding upstream of the scores flips the
  selected set and cascades through the sequential matrix GRU (rel err 0.3+).
  Layer-2's data path (h storage, gathers, S_T, both matmuls, output) is
  fp16: selection happens before rounding, everything after is smooth.
  End-to-end sim rel err: 8.8e-4.
- segment_sum linearity: segsum(w * (Z@Q)[src], dst) == segsum(w*Z[src], dst) @ Q.
  Per 128-edge subchunk the core dma_gathers 128 rows of Z, builds the
  weighted one-hot S_T[e, d] = w_e * (dst_e == d) with one fused DVE
  tensor_scalar (is_equal x mult against a constant iota), and accumulates
  G.T = X.T @ S_T in PSUM with one matmul. After a group finishes:
  out = lrelu(G @ Q) via one more matmul.
- Layer boundary: per-step AllGather of the h slices (+ device-computed
  layer-2 scores); on-device exact top-k (vector.max8/max_index + global
  rank by count + indirect rank-scatter) and the 128x128 matrix GRU produce
  layer-2's evolved weights. Layer-1's evolved weights are host-precomputed
  (tiny sequential GRU on pure inputs, replicated - see sharding hint).
"""
import os
import sys

for _p in ("/opt/trn_rl_repo", "/root/.axon_site/_ro/trn_rl_repo"):
    if os.path.isdir(_p) and _p not in sys.path:
        sys.path.insert(0, _p)

import numpy as np

# Persistent jax compilation cache: run_bass_kernel_spmd re-jits a fresh
# closure per call, so without this every invocation re-runs the full
# walrus BIR->NEFF compile (~10-40s). The cache is keyed on the HLO hash;
# the executable still loads + runs on-device per call.
import jax as _jax

for _k, _v in (("jax_compilation_cache_dir", "/tmp/jax_cc_cache"),
               ("jax_persistent_cache_min_compile_time_secs", 0.0),
               ("jax_persistent_cache_min_entry_size_bytes", 0)):
    try:
        _jax.config.update(_k, _v)
    except Exception:
        pass

import concourse.bass as bass
import concourse.bacc as bacc
import concourse.mybir as mybir
import concourse.tile as tile
from concourse.bass_utils import run_bass_kernel_spmd

F32 = mybir.dt.float32
F16 = mybir.dt.float16
I16 = mybir.dt.int16
I32 = mybir.dt.int32
I8 = mybir.dt.int8
U8 = mybir.dt.uint8
ALU = mybir.AluOpType
ACT = mybir.ActivationFunctionType
SLOPE = float((1.0 / 8.0 + 1.0 / 3.0) / 2.0)  # rrelu eval-mode slope
WSCALE = 32768.0  # edge-weight int16 fixed-point scale
NSCALE = 1024.0   # nodes 24-bit fixed-point scale (range +-32, frac 2^-18)


class Cfg:
    def __init__(self, T, N, E, ncores, gw=256, topk_rounds=2):
        self.T, self.N, self.E, self.NCORES = T, N, E, ncores
        assert N % ncores == 0
        self.NPART = N // ncores          # dst nodes per core
        self.GW = gw                      # dst group width (matmul free dim)
        self.NG = -(-self.NPART // gw)    # groups per core
        self.SPLIT = min(32768 - (32768 % 128), -(-N // 2 // 128) * 128)
        if N <= 32767:
            self.SPLIT = -(-N // 2 // 128) * 128  # exercise both halves
        self.SPLIT = min(self.SPLIT, N)
        self.D = 128
        self.K = 128
        self.C_SC = -(-N // 128)          # score columns per partition
        self.PADN = 128 * self.C_SC
        self.R = topk_rounds              # rounds of per-partition max8
        self.NCAND = 128 * 8 * topk_rounds
        self.F_GH = None                  # set from data
        self.ncol = None
        self.ncol8 = None

    def set_fgh(self, f):
        self.F_GH = f
        self.ncol = self.NG * 2 * f           # metadata columns per t
        self.ncol8 = self.NG * 2 * f * 8      # idx columns per t


# ---------------------------------------------------------------- host prep
def _pack_edges(cfg, edge_src, edge_dst, edge_w):
    """Per-core static streams. Returns (idx, dstloc, w) arrays:
    idx   [NCORES, T, 16, ncol8]  int16   (16-row wrapped band)
    dstloc[NCORES, T, 128, ncol]  float16 (values 0..GW-1, exact in fp16)
    wpack [NCORES, T, 128, ncol]  int16   (w * 32768 fixed-point)
    """
    T, NG, GW, NPART, SPLIT = cfg.T, cfg.NG, cfg.GW, cfg.NPART, cfg.SPLIT
    NC = cfg.NCORES
    # fixed subchunks per (group, half): global max over all (core,t,g,half)
    maxc = 0
    percore_key = []
    for t in range(T):
        dst, src = edge_dst[t], edge_src[t]
        key = ((dst // NPART) * NG + (dst % NPART) // GW) * 2 + (src >= SPLIT)
        percore_key.append(key.astype(np.int64))
        maxc = max(maxc, int(np.bincount(key, minlength=NC * NG * 2).max()))
    F = -(-maxc // 128)
    cfg.set_fgh(F)

    idx = np.zeros((NC, T, 16, cfg.ncol8), np.int16)
    dstloc = np.zeros((NC, T, 128, cfg.ncol), np.uint8)
    wpack = np.zeros((NC, T, 128, cfg.ncol), np.int16)
    for t in range(T):
        dst, src, w = edge_dst[t], edge_src[t], edge_w[t]
        key = percore_key[t]
        order = np.argsort(key, kind="stable")
        key_s = key[order]
        src_s, dst_s, w_s = src[order], dst[order], w[order]
        cnt = np.bincount(key_s, minlength=NC * NG * 2)
        start = np.concatenate([[0], np.cumsum(cnt)[:-1]])
        pos = np.arange(len(key_s)) - start[key_s]  # position within block
        core = key_s // (NG * 2)
        blk = key_s % (NG * 2)                      # (g*2+half) within core
        i = pos                                     # stream slot in block
        s_sub, p_row = i // 128, i % 128
        col = blk * F + s_sub
        dl = (dst_s % NPART) % GW
        half = blk % 2
        iv = src_s - half * SPLIT
        assert iv.max() < 32768 and iv.min() >= 0
        dstloc[core, t, p_row, col] = dl.astype(np.uint8)
        wpack[core, t, p_row, col] = np.minimum(
            np.round(w_s.astype(np.float64) * WSCALE), WSCALE - 1).astype(np.int16)
        # 16-row wrapped indices
        r = i % 16
        j = blk * F * 8 + i // 16
        idx[core, t, r, j] = iv.astype(np.int16)
    return idx, dstloc, wpack


def _host_gru_layer1(cfg, nodes, W_init, scorer, gW, gU, gb):
    """Exact fp32 replica of the reference layer-1 weight evolution."""
    sn = np.float32(np.linalg.norm(scorer))
    Q = W_init.copy()
    qns = []
    for t in range(cfg.T):
        Z = nodes[t]
        scores = (Z @ scorer)[:, 0] / sn
        idx = np.argsort(-scores, kind="stable")[: cfg.K]
        z_topk = (Z[idx] * np.tanh(scores[idx])[:, None]).T
        upd = 1.0 / (1.0 + np.exp(-(gW[0] @ z_topk + gU[0] @ Q + gb[0])))
        rst = 1.0 / (1.0 + np.exp(-(gW[1] @ z_topk + gU[1] @ Q + gb[1])))
        h_cap = np.tanh(gW[2] @ z_topk + gU[2] @ (rst * Q) + gb[2])
        Q = (1.0 - upd) * Q + upd * h_cap
        qns.append(Q.copy())
    return np.stack(qns).astype(np.float32)


def _geom(cfg):
    """Group geometry: list of (r0, [(row0, width<=128), ...]) per group."""
    geom = []
    for g in range(cfg.NG):
        r0 = g * cfg.GW
        r1 = min(r0 + cfg.GW, cfg.NPART)
        hh = []
        x = r0
        while x < r1:
            wdt = min(128, r1 - x)
            hh.append((x, wdt))
            x += wdt
        geom.append((r0, hh))
    return geom


# ---------------------------------------------------------------- device build
def _build(cfg):
    nc = bacc.Bacc("TRN2", target_bir_lowering=False, debug=False,
                   num_devices=cfg.NCORES)
    T, N, D, GW, NG, F, NPART = cfg.T, cfg.N, cfg.D, cfg.GW, cfg.NG, cfg.F_GH, cfg.NPART
    SPLIT, C_SC, PADN, R = cfg.SPLIT, cfg.C_SC, cfg.PADN, cfg.R
    NCAND = cfg.NCAND
    core_ids = list(range(cfg.NCORES))

    def dram_in(name, shape, dtype=F32):
        return nc.dram_tensor(name, list(shape), dtype, kind="ExternalInput").ap()

    nhi_d = dram_in("nhi", (T, NPART, D), I16)   # round(z*1024)
    nlo_d = dram_in("nlo", (T, NPART, D), U8)    # frac plane: (resid+.5)*256
    qn1 = dram_in("qn1", (T, D, D))
    gWT2 = dram_in("gWT2", (3, D, D))
    gUT2 = dram_in("gUT2", (3, D, D))
    gb2 = dram_in("gb2", (3, D, D))
    winit2 = dram_in("winit2", (D, D))
    scorer2 = dram_in("scorer2", (D, 1))          # pre-normalized
    iota_gw = dram_in("iota_gw", (128, GW))       # row = 0..GW-1, all partitions
    iota_col = dram_in("iota_col", (128, 1))      # p * C_SC
    identity = dram_in("identity", (128, 128))
    negpad = dram_in("negpad", (1, 128))          # -1e30 row
    idx_d = dram_in("idx", (T, 16, cfg.ncol8), I16)
    dstloc_d = dram_in("dstloc", (T, 128, cfg.ncol), U8)
    w_d = dram_in("wv", (T, 128, cfg.ncol), I16)

    geom = _geom(cfg)
    NBLK = sum(len(hh) for _, hh in geom)
    # int8 output, transposed [D, NPART] + per-(feature, row-block) amax scales
    out_d = nc.dram_tensor("out", [T, D, NPART], I8, kind="ExternalOutput").ap()
    scales_d = nc.dram_tensor("scales", [T, D, NBLK], F32,
                              kind="ExternalOutput").ap()

    with tile.TileContext(nc) as tc:
        import contextlib
        ctx = contextlib.ExitStack()
        with ctx:
            sb = ctx.enter_context(tc.tile_pool(name="sb", bufs=1))
            meta = ctx.enter_context(tc.tile_pool(name="meta", bufs=1))
            rcp = ctx.enter_context(tc.tile_pool(name="rcp", bufs=1))
            xgp = ctx.enter_context(tc.tile_pool(name="xgp", bufs=3))
            stp = ctx.enter_context(tc.tile_pool(name="stp", bufs=8))
            gtp = ctx.enter_context(tc.tile_pool(name="gtp", bufs=3))
            drp = ctx.enter_context(tc.tile_pool(name="drp", bufs=4))
            psg = ctx.enter_context(tc.tile_pool(name="psg", bufs=2, space="PSUM"))
            pso = ctx.enter_context(tc.tile_pool(name="pso", bufs=1, space="PSUM"))
            tkp = ctx.enter_context(tc.tile_pool(name="tkp", bufs=1))
            dram = ctx.enter_context(tc.tile_pool(name="dram", bufs=1, space="DRAM"))

            # constants
            iota_sb = sb.tile([128, GW], F32, tag="iota")
            nc.sync.dma_start(out=iota_sb[:], in_=iota_gw[:])
            ident_sb = sb.tile([128, 128], F32, tag="ident")
            nc.sync.dma_start(out=ident_sb[:], in_=identity[:])
            iotac_sb = sb.tile([128, 1], F32, tag="iotac")
            nc.sync.dma_start(out=iotac_sb[:], in_=iota_col[:])
            neg_sb = sb.tile([1, 128], F32, tag="negp")
            nc.sync.dma_start(out=neg_sb[:], in_=negpad[:])
            sc2_sb = sb.tile([128, 1], F32, tag="sc2")
            nc.sync.dma_start(out=sc2_sb[:], in_=scorer2[:])
            gW_sb, gU_sb, gb_sb = [], [], []
            for i in range(3):
                a = sb.tile([128, 128], F32, name=f"gw{i}", tag=f"gw{i}")
                nc.sync.dma_start(out=a[:], in_=gWT2[i])
                gW_sb.append(a)
                b = sb.tile([128, 128], F32, name=f"gu{i}", tag=f"gu{i}")
                nc.sync.dma_start(out=b[:], in_=gUT2[i])
                gU_sb.append(b)
                c = sb.tile([128, 128], F32, name=f"gb{i}", tag=f"gb{i}")
                nc.sync.dma_start(out=c[:], in_=gb2[i])
                gb_sb.append(c)
            qn1_sb = []
            for t in range(T):
                q = sb.tile([128, 128], F32, name=f"qn1_{t}", tag=f"qn1_{t}")
                nc.sync.dma_start(out=q[:], in_=qn1[t])
                qn1_sb.append(q)

            # persistent DRAM buffers
            nodes_sl = [dram.tile([NPART, D], F32, name=f"nsl{t}", tag=f"nsl{t}")
                        for t in range(T)]
            nodes_full = [dram.tile([N, D], F32, name=f"nfl{t}", tag=f"nfl{t}",
                                    addr_space="Shared") for t in range(T)]
            nodes_loc = [dram.tile([N, D], F32, name=f"nlc{t}", tag=f"nlc{t}")
                         for t in range(T)]
            h_slice = [dram.tile([NPART, D], F16, name=f"hsl{t}", tag=f"hsl{t}")
                       for t in range(T)]
            h_full = [dram.tile([N, D], F16, name=f"hfl{t}", tag=f"hfl{t}",
                                addr_space="Shared") for t in range(T)]
            sc_slice = [dram.tile([1, NPART], F32, name=f"ssl{t}", tag=f"ssl{t}")
                        for t in range(T)]
            h_loc = [dram.tile([N, D], F16, name=f"hlc{t}", tag=f"hlc{t}")
                     for t in range(T)]
            sc_full = [dram.tile([1, PADN], F32, name=f"sfl{t}", tag=f"sfl{t}",
                                 addr_space="Shared") for t in range(T)]
            cand_dram = dram.tile([1, NCAND], F32, tag="cand", bufs=2)
            sorted_dram = dram.tile([129, 2], F32, tag="sorted", bufs=2)

            qn2_sb = [sb.tile([128, 128], F32, name=f"qn2_{t}", tag=f"qn2_{t}")
                      for t in range(T)]
            qn2h_sb = [sb.tile([128, 128], F16, name=f"qn2h_{t}", tag=f"qn2h_{t}")
                       for t in range(T)]

            # reconstruct f32 nodes shard from 24-bit planes, then AllGather:
            # z = (hi + lo/256 - 0.5) / 1024
            FLAT = NPART * D // 128          # flat columns per t (partition-major)
            NCH = 5
            CH = FLAT // NCH
            assert CH * NCH == FLAT
            for t in range(T):
                hi_flat = nhi_d[t].rearrange("a d -> (a d)").rearrange(
                    "(p c) -> p c", c=FLAT)
                lo_flat = nlo_d[t].rearrange("a d -> (a d)").rearrange(
                    "(p c) -> p c", c=FLAT)
                sl_flat = nodes_sl[t][:].rearrange("a d -> (a d)").rearrange(
                    "(p c) -> p c", c=FLAT)
                for k in range(NCH):
                    cs = slice(k * CH, (k + 1) * CH)
                    rhi = rcp.tile([128, CH], I16, tag="rhi")
                    nc.sync.dma_start(out=rhi[:], in_=hi_flat[:, cs])
                    rlo = rcp.tile([128, CH], U8, tag="rlo")
                    nc.sync.dma_start(out=rlo[:], in_=lo_flat[:, cs])
                    rhf = rcp.tile([128, CH], F32, tag="rhf")
                    nc.vector.tensor_copy(out=rhf[:], in_=rhi[:])
                    rlf = rcp.tile([128, CH], F32, tag="rlf")
                    nc.vector.tensor_copy(out=rlf[:], in_=rlo[:])
                    rt1 = rcp.tile([128, CH], F32, tag="rt1")
                    nc.vector.tensor_scalar(out=rt1[:], in0=rlf[:],
                                            scalar1=float(1.0 / 256.0),
                                            scalar2=-0.5,
                                            op0=ALU.mult, op1=ALU.add)
                    rt2 = rcp.tile([128, CH], F32, tag="rt2")
                    nc.vector.tensor_tensor(out=rt2[:], in0=rhf[:], in1=rt1[:],
                                            op=ALU.add)
                    rz = rcp.tile([128, CH], F32, tag="rz")
                    nc.vector.tensor_scalar(out=rz[:], in0=rt2[:],
                                            scalar1=float(1.0 / NSCALE),
                                            scalar2=None, op0=ALU.mult)
                    nc.sync.dma_start(out=sl_flat[:, cs], in_=rz[:])
                nc.gpsimd.collective_compute(
                    "AllGather", ALU.bypass,
                    replica_groups=[core_ids],
                    ins=[nodes_sl[t][:].opt()],
                    outs=[nodes_full[t][:].opt()])
                nc.sync.dma_start(out=nodes_loc[t][:], in_=nodes_full[t][:])

            def spmm_pass(t, z_src_ap, qn_tile, layer):
                """One (layer, t) SpMM pass. z_src_ap: [N, D] DRAM AP
                (f32 for layer 1, fp16 for layer 2)."""
                zdt = F32 if layer == 1 else F16
                idx_sb = meta.tile([128, cfg.ncol8], I16, tag="idx")
                for s in range(8):
                    nc.sync.dma_start(out=idx_sb[16 * s:16 * (s + 1), :],
                                      in_=idx_d[t])
                dl8_sb = meta.tile([128, cfg.ncol], U8, tag="dl8")
                nc.sync.dma_start(out=dl8_sb[:], in_=dstloc_d[t])
                dl_sb = meta.tile([128, cfg.ncol], F32, tag="dl")
                nc.vector.tensor_copy(out=dl_sb[:], in_=dl8_sb[:])
                wq_sb = meta.tile([128, cfg.ncol], I16, tag="wq")
                nc.sync.dma_start(out=wq_sb[:], in_=w_d[t])
                w_sb = meta.tile([128, cfg.ncol], F32, tag="wv")
                nc.vector.tensor_copy(out=w_sb[:], in_=wq_sb[:])
                z_lo = z_src_ap[0:SPLIT, :]
                z_hi = z_src_ap[SPLIT:N, :]
                if layer == 2:
                    sc8 = gtp.tile([128, NBLK], F32, tag="sc8")
                bi = 0
                for g in range(NG):
                    r0, hh = geom[g]
                    xg = []
                    for half, zsrc in ((0, z_lo), (1, z_hi)):
                        xt = xgp.tile([128, F * 128], zdt, tag="xg",
                                      name=f"xg{layer}_{t}_{g}_{half}")
                        c0 = (g * 2 + half) * F * 8
                        # single_packet SWDGE limit: <=64 desc/engine -> 1024 idxs
                        for s0 in range(0, F, 8):
                            ns = min(8, F - s0)
                            nc.gpsimd.dma_gather(
                                out_ap=xt[:, s0 * 128:(s0 + ns) * 128]
                                .rearrange("p (s e) -> p s e", e=128),
                                in_ap=zsrc,
                                idxs_ap=idx_sb[:, c0 + s0 * 8:c0 + (s0 + ns) * 8],
                                num_idxs=ns * 128,
                                num_idxs_reg=ns * 128,
                                elem_size=128,
                            )
                        xg.append(xt)
                    gt_ps = psg.tile([128, GW], F32, tag="gt", space="PSUM")
                    nmm = 2 * F
                    k = 0
                    for half in (0, 1):
                        for s in range(F):
                            col = (g * 2 + half) * F + s
                            st = stp.tile([128, GW], zdt, tag="st",
                                          name=f"st{layer}_{t}_{g}_{half}_{s}")
                            nc.vector.tensor_scalar(
                                out=st[:], in0=iota_sb[:],
                                scalar1=dl_sb[:, col:col + 1],
                                scalar2=w_sb[:, col:col + 1],
                                op0=ALU.is_equal, op1=ALU.mult)
                            lhs = xg[half][:, s * 128:(s + 1) * 128]
                            nc.tensor.matmul(out=gt_ps[:], lhsT=lhs, rhs=st[:],
                                             start=(k == 0), stop=(k == nmm - 1))
                            k += 1
                    # copy-out descales the int16 fixed-point edge weights
                    gt_sb = gtp.tile([128, GW], zdt, tag="gts")
                    nc.scalar.activation(out=gt_sb[:], in_=gt_ps[:], func=ACT.Copy,
                                         scale=float(1.0 / WSCALE))
                    for (rr, wdt) in hh:
                        o_ps = pso.tile([128, 128], F32, tag="ops", space="PSUM", bufs=2)
                        lhs2 = gt_sb[:, rr - r0:rr - r0 + wdt]
                        rhs2 = qn_tile[:]
                        nc.tensor.matmul(out=o_ps[:wdt, :], lhsT=lhs2, rhs=rhs2,
                                         start=True, stop=True)
                        sx = drp.tile([128, 128], F32, tag="sx")
                        nc.scalar.activation(out=sx[:wdt, :], in_=o_ps[:wdt, :],
                                             func=ACT.Copy, scale=SLOPE)
                        hb = drp.tile([128, 128], F32, tag="hb")
                        nc.vector.tensor_tensor(out=hb[:wdt, :], in0=o_ps[:wdt, :],
                                                in1=sx[:wdt, :], op=ALU.max)
                        # both layers transpose h (layer 1: scores; layer 2:
                        # per-feature int8 quantization on partitions)
                        ht_ps = pso.tile([128, 128], F32, tag="htp",
                                         space="PSUM")
                        nc.tensor.transpose(out=ht_ps[:, :wdt], in_=hb[:wdt, :],
                                            identity=ident_sb[:wdt, :wdt])
                        ht_sb = drp.tile([128, 128], F32, tag="hts")
                        nc.scalar.activation(out=ht_sb[:, :wdt],
                                             in_=ht_ps[:, :wdt], func=ACT.Copy)
                        if layer == 1:
                            hb16 = drp.tile([128, 128], F16, tag="hb16")
                            nc.vector.tensor_copy(out=hb16[:wdt, :],
                                                  in_=hb[:wdt, :])
                            nc.sync.dma_start(out=h_slice[t][rr:rr + wdt, :],
                                              in_=hb16[:wdt, :])
                            s_ps = pso.tile([1, 128], F32, tag="sps", space="PSUM")
                            nc.tensor.matmul(out=s_ps[:, :wdt], lhsT=sc2_sb[:],
                                             rhs=ht_sb[:, :wdt], start=True,
                                             stop=True)
                            s_sb = drp.tile([1, 128], F32, tag="ssb")
                            nc.scalar.activation(out=s_sb[:, :wdt],
                                                 in_=s_ps[:, :wdt], func=ACT.Copy)
                            nc.sync.dma_start(
                                out=sc_slice[t][:, rr:rr + wdt],
                                in_=s_sb[:1, :wdt])
                        else:
                            # int8 quantize per feature row of ht
                            mx = drp.tile([128, 1], F32, tag="qmx")
                            nc.vector.tensor_reduce(
                                out=mx[:], in_=ht_sb[:, :wdt],
                                axis=mybir.AxisListType.X, op=ALU.max)
                            mn = drp.tile([128, 1], F32, tag="qmn")
                            nc.vector.tensor_reduce(
                                out=mn[:], in_=ht_sb[:, :wdt],
                                axis=mybir.AxisListType.X, op=ALU.min)
                            nmn = drp.tile([128, 1], F32, tag="qnm")
                            nc.vector.tensor_scalar(out=nmn[:], in0=mn[:],
                                                    scalar1=-1.0, scalar2=None,
                                                    op0=ALU.mult)
                            am = drp.tile([128, 1], F32, tag="qam")
                            nc.vector.tensor_tensor(out=am[:], in0=mx[:],
                                                    in1=nmn[:], op=ALU.max)
                            amc = drp.tile([128, 1], F32, tag="qac")
                            nc.vector.tensor_scalar(out=amc[:], in0=am[:],
                                                    scalar1=1e-30, scalar2=None,
                                                    op0=ALU.max)
                            rc = drp.tile([128, 1], F32, tag="qrc")
                            nc.vector.reciprocal(out=rc[:], in_=amc[:])
                            inv = drp.tile([128, 1], F32, tag="qin")
                            nc.vector.tensor_scalar(out=inv[:], in0=rc[:],
                                                    scalar1=127.0, scalar2=None,
                                                    op0=ALU.mult)
                            q8 = drp.tile([128, 128], I8, tag="q8")
                            nc.vector.tensor_scalar(out=q8[:, :wdt],
                                                    in0=ht_sb[:, :wdt],
                                                    scalar1=inv[:, 0:1],
                                                    scalar2=None, op0=ALU.mult)
                            nc.vector.tensor_copy(out=sc8[:, bi:bi + 1],
                                                  in_=amc[:])
                            nc.sync.dma_start(out=out_d[t, :, rr:rr + wdt],
                                              in_=q8[:, :wdt])
                        bi += 1
                if layer == 2:
                    nc.sync.dma_start(out=scales_d[t], in_=sc8[:])
                if layer == 1:
                    nc.gpsimd.collective_compute(
                        "AllGather", ALU.bypass,
                        replica_groups=[core_ids],
                        ins=[h_slice[t][:].opt()],
                        outs=[h_full[t][:].opt()])
                    nc.sync.dma_start(out=h_loc[t][:], in_=h_full[t][:])
                    nc.gpsimd.collective_compute(
                        "AllGather", ALU.bypass,
                        replica_groups=[core_ids],
                        ins=[sc_slice[t][:].opt()],
                        outs=[sc_full[t][:, 0:N].opt()])

            def topk_gru(t, q_prev):
                """Exact top-128 of sc_full[t] + matrix GRU -> qn2_sb[t]."""
                S = tkp.tile([128, C_SC], F32, tag="S")
                nc.sync.dma_start(out=S[:],
                                  in_=sc_full[t][:].rearrange("o (p c) -> (o p) c",
                                                              c=C_SC))
                if PADN > N:
                    p_t, c_t = N // C_SC, N % C_SC
                    nc.sync.dma_start(out=S[p_t:p_t + 1, c_t:C_SC],
                                      in_=negpad[0:1, 0:C_SC - c_t])
                    if p_t + 1 < 128:
                        nc.sync.dma_start(
                            out=S[p_t + 1:128, :],
                            in_=negpad[0:1, 0:1].to_broadcast(
                                [127 - p_t, C_SC]))
                vals = tkp.tile([128, 8 * R], F32, tag="vals")
                cols = tkp.tile([128, 8 * R], F32, tag="cols")
                Swork = S
                for r in range(R):
                    mx = tkp.tile([128, 8], F32, tag="mx")
                    nc.vector.max(out=mx[:], in_=Swork[:])
                    ix = tkp.tile([128, 8], mybir.dt.uint32, tag="ix")
                    nc.vector.max_index(out=ix[:], in_max=mx[:], in_values=Swork[:])
                    nc.vector.tensor_copy(out=vals[:, r * 8:(r + 1) * 8], in_=mx[:])
                    nc.vector.tensor_copy(out=cols[:, r * 8:(r + 1) * 8], in_=ix[:])
                    if r < R - 1:
                        S2 = tkp.tile([128, C_SC], F32, tag=f"Sw{r % 2}")
                        nc.vector.match_replace(out=S2[:], in_to_replace=mx[:],
                                                in_values=Swork[:],
                                                imm_value=-1e30)
                        Swork = S2
                # global node id n = p*C_SC + col
                nid = tkp.tile([128, 8 * R], F32, tag="nid")
                nc.vector.tensor_scalar(out=nid[:], in0=cols[:],
                                        scalar1=iotac_sb[:, :1], scalar2=None,
                                        op0=ALU.add)
                # broadcast all candidates to all partitions via DRAM bounce
                nc.sync.dma_start(out=cand_dram[:], in_=vals[:])
                cb = tkp.tile([128, NCAND], F32, tag="cb")
                nc.sync.dma_start(out=cb[:],
                                  in_=cand_dram[:].to_broadcast([128, NCAND]))
                rank = tkp.tile([128, 8 * R], F32, tag="rank")
                for j in range(8 * R):
                    cmp = tkp.tile([128, NCAND], F32, tag="cmp")
                    nc.vector.tensor_scalar(out=cmp[:], in0=cb[:],
                                            scalar1=vals[:, j:j + 1], scalar2=None,
                                            op0=ALU.is_gt)
                    nc.vector.tensor_reduce(out=rank[:, j:j + 1], in_=cmp[:],
                                            axis=mybir.AxisListType.X, op=ALU.add)
                nc.vector.tensor_scalar(out=rank[:], in0=rank[:], scalar1=128.0,
                                        scalar2=None, op0=ALU.min)
                ri = tkp.tile([128, 8 * R], I32, tag="ri")
                nc.vector.tensor_copy(out=ri[:], in_=rank[:])
                pairs = tkp.tile([128, 16 * R], F32, tag="pairs")
                nc.vector.tensor_copy(
                    out=pairs[:].rearrange("p (j two) -> p j two", two=2)[:, :, 0],
                    in_=nid[:])
                nc.vector.tensor_copy(
                    out=pairs[:].rearrange("p (j two) -> p j two", two=2)[:, :, 1],
                    in_=vals[:])
                for j in range(8 * R):
                    nc.gpsimd.indirect_dma_start(
                        out=sorted_dram[:],
                        out_offset=bass.IndirectOffsetOnAxis(
                            ap=ri[:, j:j + 1], axis=0),
                        in_=pairs[:, 2 * j:2 * j + 2],
                        in_offset=None)
                sv = tkp.tile([128, 2], F32, tag="sv")
                nc.sync.dma_start(out=sv[:], in_=sorted_dram[0:128, :])
                nidx = tkp.tile([128, 1], I32, tag="nidx")
                nc.vector.tensor_copy(out=nidx[:], in_=sv[:, 0:1])
                tanhv = tkp.tile([128, 1], F32, tag="tanhv")
                nc.scalar.activation(out=tanhv[:], in_=sv[:, 1:2], func=ACT.Tanh)
                zsel16 = tkp.tile([128, 128], F16, tag="zsel16")
                nc.gpsimd.indirect_dma_start(
                    out=zsel16[:], out_offset=None,
                    in_=h_full[t][:],
                    in_offset=bass.IndirectOffsetOnAxis(ap=nidx[:, :1], axis=0))
                zsel = tkp.tile([128, 128], F32, tag="zsel")
                nc.vector.tensor_copy(out=zsel[:], in_=zsel16[:])
                zs2 = tkp.tile([128, 128], F32, tag="zs2")
                nc.scalar.activation(out=zs2[:], in_=zsel[:], func=ACT.Copy,
                                     scale=tanhv[:, :1])
                zt_ps = pso.tile([128, 128], F32, tag="ztp", space="PSUM")
                nc.tensor.transpose(out=zt_ps[:], in_=zs2[:], identity=ident_sb[:])
                ztop = tkp.tile([128, 128], F32, tag="ztop")
                nc.scalar.activation(out=ztop[:], in_=zt_ps[:], func=ACT.Copy)
                # matrix GRU
                gates = []
                rstq = None
                for i in range(3):
                    g_ps = pso.tile([128, 128], F32, tag="gps", space="PSUM")
                    nc.tensor.matmul(out=g_ps[:], lhsT=gW_sb[i][:], rhs=ztop[:],
                                     start=True, stop=False)
                    other = q_prev if i < 2 else rstq
                    nc.tensor.matmul(out=g_ps[:], lhsT=gU_sb[i][:], rhs=other[:],
                                     start=False, stop=True)
                    gsum = tkp.tile([128, 128], F32, tag=f"gsum{i}")
                    nc.vector.tensor_tensor(out=gsum[:], in0=g_ps[:],
                                            in1=gb_sb[i][:], op=ALU.add)
                    gact = tkp.tile([128, 128], F32, tag=f"gact{i}")
                    nc.scalar.activation(out=gact[:], in_=gsum[:],
                                         func=(ACT.Sigmoid if i < 2 else ACT.Tanh))
                    gates.append(gact)
                    if i == 1:
                        rstq = tkp.tile([128, 128], F32, tag="rstq")
                        nc.vector.tensor_tensor(out=rstq[:], in0=gates[1][:],
                                                in1=q_prev[:], op=ALU.mult)
                upd, h_cap = gates[0], gates[2]
                dql = tkp.tile([128, 128], F32, tag="dql")
                nc.vector.tensor_tensor(out=dql[:], in0=h_cap[:], in1=q_prev[:],
                                        op=ALU.subtract)
                udl = tkp.tile([128, 128], F32, tag="udl")
                nc.vector.tensor_tensor(out=udl[:], in0=upd[:], in1=dql[:],
                                        op=ALU.mult)
                nc.vector.tensor_tensor(out=qn2_sb[t][:], in0=q_prev[:],
                                        in1=udl[:], op=ALU.add)
                nc.vector.tensor_copy(out=qn2h_sb[t][:], in_=qn2_sb[t][:])
                return qn2_sb[t]

            # ---- program ----
            bisect = os.environ.get("KBISECT", "")
            if bisect.startswith("spmm"):
                npass = int(bisect[4:] or 2 * T)
                for i in range(npass):
                    spmm_pass(i % T, nodes_loc[i % T][:], qn1_sb[i % T], layer=1)
            else:
                for t in range(T):
                    spmm_pass(t, nodes_loc[t][:], qn1_sb[t], layer=1)
                qprev = sb.tile([128, 128], F32, name="winit2_sb", tag="winit2")
                nc.sync.dma_start(out=qprev[:], in_=winit2[:])
                for t in range(T):
                    qprev = topk_gru(t, qprev)
                for t in range(T):
                    spmm_pass(t, h_loc[t][:], qn2h_sb[t], layer=2)

    nc.compile()
    return nc


# ---------------------------------------------------------------- entry point
_CACHE = {}
_LAST_IN_MAPS = None
_LAST_CFG = None

# full-problem constants (hardcoded per contract)
_T, _N, _E, _NCORES = 6, 50000, 1600000, 8


def _pack_nodes_24bit(zs):
    """[.., ] f32 -> (int16 hi, uint8 lo): z ~ (hi + lo/256 - 0.5)/1024."""
    s = zs.astype(np.float64) * NSCALE
    hi = np.round(s)
    assert np.abs(hi).max() < 32767, "nodes exceed 24-bit fixed-point range"
    lo = np.clip(np.round((s - hi + 0.5) * 256.0), 0, 255)
    return hi.astype(np.int16), lo.astype(np.uint8)


def assemble_out(res, cfg=None):
    """Dequantize per-core int8 outputs -> full [T, N, D] f32."""
    cfg = cfg or _LAST_CFG
    geom = _geom(cfg)
    bi_of_row = np.zeros(cfg.NPART, np.int64)
    bi = 0
    for _, hh in geom:
        for (rr, wdt) in hh:
            bi_of_row[rr:rr + wdt] = bi
            bi += 1
    outs = []
    for c in range(cfg.NCORES):
        q = res.results[c]["out"].astype(np.float32)    # [T, D, NPART]
        am = res.results[c]["scales"]                   # [T, D, NBLK]
        amr = am[:, :, bi_of_row]                       # [T, D, NPART]
        outs.append(np.transpose(q * (amr * (1.0 / 127.0)), (0, 2, 1)))
    return np.concatenate(outs, axis=1).astype(np.float32)


def kernel(nodes, edge_src, edge_dst, edge_weight,
           W_init1, scorer1, gate_W1, gate_U1, gate_b1,
           W_init2, scorer2, gate_W2, gate_U2, gate_b2):
    nodes = np.ascontiguousarray(np.asarray(nodes, np.float32))
    T, N, D = nodes.shape
    E = np.asarray(edge_src).shape[1]
    gw = int(os.environ.get("KGW", "256"))
    cfg = Cfg(T, N, E, _NCORES, gw=gw, topk_rounds=2)
    idx, dstloc, wpack = _pack_edges(
        cfg, np.asarray(edge_src), np.asarray(edge_dst),
        np.asarray(edge_weight, np.float32))
    qn1 = _host_gru_layer1(cfg, nodes, np.asarray(W_init1, np.float32),
                           np.asarray(scorer1, np.float32),
                           np.asarray(gate_W1, np.float32),
                           np.asarray(gate_U1, np.float32),
                           np.asarray(gate_b1, np.float32))
    key = (T, N, E, cfg.F_GH, cfg.GW, cfg.R)
    if key not in _CACHE:
        _CACHE[key] = _build(cfg)
    nc = _CACHE[key]

    sc2n = (np.asarray(scorer2, np.float32)
            / np.float32(np.linalg.norm(scorer2))).astype(np.float32)
    shared = {
        "qn1": qn1,
        "gWT2": np.ascontiguousarray(
            np.transpose(np.asarray(gate_W2, np.float32), (0, 2, 1))),
        "gUT2": np.ascontiguousarray(
            np.transpose(np.asarray(gate_U2, np.float32), (0, 2, 1))),
        "gb2": np.asarray(gate_b2, np.float32),
        "winit2": np.asarray(W_init2, np.float32),
        "scorer2": sc2n,
        "iota_gw": np.tile(np.arange(cfg.GW, dtype=np.float32), (128, 1)),
        "iota_col": (np.arange(128, dtype=np.float32) * cfg.C_SC)[:, None],
        "identity": np.eye(128, dtype=np.float32),
        "negpad": np.full((1, 128), -1e30, np.float32),
    }
    in_maps = []
    for c in range(cfg.NCORES):
        m = dict(shared)
        nhi, nlo = _pack_nodes_24bit(
            nodes[:, c * cfg.NPART:(c + 1) * cfg.NPART, :])
        m["nhi"] = nhi
        m["nlo"] = nlo
        m["idx"] = idx[c]
        m["dstloc"] = dstloc[c]
        m["wv"] = wpack[c]
        in_maps.append(m)
    global _LAST_IN_MAPS, _LAST_CFG
    _LAST_IN_MAPS = in_maps
    _LAST_CFG = cfg
    res = run_bass_kernel_spmd(nc, in_maps, list(range(cfg.NCORES)))
    return assemble_out(res, cfg)


# revision 19
# speedup vs baseline: 4.0705x; 1.5054x over previous
"""EvolveGCN (EGCN-H, 2 GRCU layers) Trainium2 Bass kernel, 8-way SPMD.

Strategy (dst-sharded graph parallel, transfer-optimized):
- 8 cores each own a contiguous range of N/8 destination nodes. Edges are
  routed (host-side) to their dst-owner core, grouped by 256-wide dst groups
  and by src half (int16 gather-index limit), padded to a fixed per-group
  subchunk count so the device program is static and identical on all cores.
- Host->device traffic is minimized (the axon tunnel is ~20MB/s):
  nodes are uploaded f32 but SHARDED (N/8 rows per core) and AllGathered
  on-device over NeuronLink; gather indices ship as the 16-row wrapped band
  (replicated to 128 partitions on-device); dst-locations ship fp16;
  edge weights ship int16 fixed-point (w*32768, descaled for free via the
  PSUM->SBUF copy scale); h and the output ship/store fp16.
- Precision split (validated on the exact instance): layer-1 SpMM stays f32
  because the layer-2 top-k selection scores derive from h and the rank-128
  score gap is ~5e-3 - any fp16 rounding upstream of the scores flips the
  selected set and cascades through the sequential matrix GRU (rel err 0.3+).
  Layer-2's data path (h storage, gathers, S_T, both matmuls, output) is
  fp16: selection happens before rounding, everything after is smooth.
  End-to-end sim rel err: 8.8e-4.
- segment_sum linearity: segsum(w * (Z@Q)[src], dst) == segsum(w*Z[src], dst) @ Q.
  Per 128-edge subchunk the core dma_gathers 128 rows of Z, builds the
  weighted one-hot S_T[e, d] = w_e * (dst_e == d) with one fused DVE
  tensor_scalar (is_equal x mult against a constant iota), and accumulates
  G.T = X.T @ S_T in PSUM with one matmul. After a group finishes:
  out = lrelu(G @ Q) via one more matmul.
- Layer boundary: per-step AllGather of the h slices (+ device-computed
  layer-2 scores); on-device exact top-k (vector.max8/max_index + global
  rank by count + indirect rank-scatter) and the 128x128 matrix GRU produce
  layer-2's evolved weights. Layer-1's evolved weights are host-precomputed
  (tiny sequential GRU on pure inputs, replicated - see sharding hint).
"""
import os
import sys

for _p in ("/opt/trn_rl_repo", "/root/.axon_site/_ro/trn_rl_repo"):
    if os.path.isdir(_p) and _p not in sys.path:
        sys.path.insert(0, _p)

import numpy as np

# Persistent jax compilation cache: run_bass_kernel_spmd re-jits a fresh
# closure per call, so without this every invocation re-runs the full
# walrus BIR->NEFF compile (~10-40s). The cache is keyed on the HLO hash;
# the executable still loads + runs on-device per call.
import jax as _jax

for _k, _v in (("jax_compilation_cache_dir", "/tmp/jax_cc_cache"),
               ("jax_persistent_cache_min_compile_time_secs", 0.0),
               ("jax_persistent_cache_min_entry_size_bytes", 0)):
    try:
        _jax.config.update(_k, _v)
    except Exception:
        pass

import concourse.bass as bass
import concourse.bacc as bacc
import concourse.mybir as mybir
import concourse.tile as tile
from concourse.bass_utils import run_bass_kernel_spmd

F32 = mybir.dt.float32
F16 = mybir.dt.float16
I16 = mybir.dt.int16
I32 = mybir.dt.int32
I8 = mybir.dt.int8
U8 = mybir.dt.uint8
ALU = mybir.AluOpType
ACT = mybir.ActivationFunctionType
SLOPE = float((1.0 / 8.0 + 1.0 / 3.0) / 2.0)  # rrelu eval-mode slope
WSCALE = 32768.0  # edge-weight int16 fixed-point scale
NSCALE = 1024.0   # nodes 24-bit fixed-point scale (range +-32, frac 2^-18)


class Cfg:
    def __init__(self, T, N, E, ncores, gw=256, topk_rounds=2):
        self.T, self.N, self.E, self.NCORES = T, N, E, ncores
        assert N % ncores == 0
        self.NPART = N // ncores          # dst nodes per core
        self.GW = gw                      # dst group width (matmul free dim)
        self.NG = -(-self.NPART // gw)    # groups per core
        self.SPLIT = min(32768 - (32768 % 128), -(-N // 2 // 128) * 128)
        if N <= 32767:
            self.SPLIT = -(-N // 2 // 128) * 128  # exercise both halves
        self.SPLIT = min(self.SPLIT, N)
        self.D = 128
        self.K = 128
        self.C_SC = -(-N // 128)          # score columns per partition
        self.PADN = 128 * self.C_SC
        self.R = topk_rounds              # rounds of per-partition max8
        self.NCAND = 128 * 8 * topk_rounds
        self.F_GH = None                  # set from data
        self.ncol = None
        self.ncol8 = None

    def set_fgh(self, f):
        self.F_GH = f
        self.ncol = self.NG * 2 * f           # metadata columns per t
        self.ncol8 = self.NG * 2 * f * 8      # idx columns per t


# ---------------------------------------------------------------- host prep
def _pack_edges(cfg, edge_src, edge_dst, edge_w):
    """Per-core static streams. Returns (idx, dstloc, w) arrays:
    idx   [NCORES, T, 16, ncol8]  int16   (16-row wrapped band)
    dstloc[NCORES, T, 128, ncol]  float16 (values 0..GW-1, exact in fp16)
    wpack [NCORES, T, 128, ncol]  int16   (w * 32768 fixed-point)
    """
    T, NG, GW, NPART, SPLIT = cfg.T, cfg.NG, cfg.GW, cfg.NPART, cfg.SPLIT
    NC = cfg.NCORES
    # fixed subchunks per (group, half): global max over all (core,t,g,half)
    maxc = 0
    percore_key = []
    for t in range(T):
        dst, src = edge_dst[t], edge_src[t]
        key = ((dst // NPART) * NG + (dst % NPART) // GW) * 2 + (src >= SPLIT)
        percore_key.append(key.astype(np.int64))
        maxc = max(maxc, int(np.bincount(key, minlength=NC * NG * 2).max()))
    F = -(-maxc // 128)
    cfg.set_fgh(F)

    idx = np.zeros((NC, T, 16, cfg.ncol8), np.int16)
    dstloc = np.zeros((NC, T, 128, cfg.ncol), np.uint8)
    wpack = np.zeros((NC, T, 128, cfg.ncol), np.int16)
    for t in range(T):
        dst, src, w = edge_dst[t], edge_src[t], edge_w[t]
        key = percore_key[t]
        order = np.argsort(key, kind="stable")
        key_s = key[order]
        src_s, dst_s, w_s = src[order], dst[order], w[order]
        cnt = np.bincount(key_s, minlength=NC * NG * 2)
        start = np.concatenate([[0], np.cumsum(cnt)[:-1]])
        pos = np.arange(len(key_s)) - start[key_s]  # position within block
        core = key_s // (NG * 2)
        blk = key_s % (NG * 2)                      # (g*2+half) within core
        i = pos                                     # stream slot in block
        s_sub, p_row = i // 128, i % 128
        col = blk * F + s_sub
        dl = (dst_s % NPART) % GW
        half = blk % 2
        iv = src_s - half * SPLIT
        assert iv.max() < 32768 and iv.min() >= 0
        dstloc[core, t, p_row, col] = dl.astype(np.uint8)
        wpack[core, t, p_row, col] = np.minimum(
            np.round(w_s.astype(np.float64) * WSCALE), WSCALE - 1).astype(np.int16)
        # 16-row wrapped indices
        r = i % 16
        j = blk * F * 8 + i // 16
        idx[core, t, r, j] = iv.astype(np.int16)
    return idx, dstloc, wpack


def _host_gru_layer1(cfg, nodes, W_init, scorer, gW, gU, gb):
    """Exact fp32 replica of the reference layer-1 weight evolution."""
    sn = np.float32(np.linalg.norm(scorer))
    Q = W_init.copy()
    qns = []
    for t in range(cfg.T):
        Z = nodes[t]
        scores = (Z @ scorer)[:, 0] / sn
        idx = np.argsort(-scores, kind="stable")[: cfg.K]
        z_topk = (Z[idx] * np.tanh(scores[idx])[:, None]).T
        upd = 1.0 / (1.0 + np.exp(-(gW[0] @ z_topk + gU[0] @ Q + gb[0])))
        rst = 1.0 / (1.0 + np.exp(-(gW[1] @ z_topk + gU[1] @ Q + gb[1])))
        h_cap = np.tanh(gW[2] @ z_topk + gU[2] @ (rst * Q) + gb[2])
        Q = (1.0 - upd) * Q + upd * h_cap
        qns.append(Q.copy())
    return np.stack(qns).astype(np.float32)


def _geom(cfg):
    """Group geometry: list of (r0, [(row0, width<=128), ...]) per group."""
    geom = []
    for g in range(cfg.NG):
        r0 = g * cfg.GW
        r1 = min(r0 + cfg.GW, cfg.NPART)
        hh = []
        x = r0
        while x < r1:
            wdt = min(128, r1 - x)
            hh.append((x, wdt))
            x += wdt
        geom.append((r0, hh))
    return geom


# ---------------------------------------------------------------- device build
def _build(cfg):
    nc = bacc.Bacc("TRN2", target_bir_lowering=False, debug=False,
                   num_devices=cfg.NCORES)
    T, N, D, GW, NG, F, NPART = cfg.T, cfg.N, cfg.D, cfg.GW, cfg.NG, cfg.F_GH, cfg.NPART
    SPLIT, C_SC, PADN, R = cfg.SPLIT, cfg.C_SC, cfg.PADN, cfg.R
    NCAND = cfg.NCAND
    core_ids = list(range(cfg.NCORES))

    def dram_in(name, shape, dtype=F32):
        return nc.dram_tensor(name, list(shape), dtype, kind="ExternalInput").ap()

    nhi_d = dram_in("nhi", (T, NPART, D), I16)   # round(z*1024)
    nlo_d = dram_in("nlo", (T, NPART, D), U8)    # frac plane: (resid+.5)*256
    qn1 = dram_in("qn1", (T, D, D))
    gWT2 = dram_in("gWT2", (3, D, D))
    gUT2 = dram_in("gUT2", (3, D, D))
    gb2 = dram_in("gb2", (3, D, D))
    winit2 = dram_in("winit2", (D, D))
    scorer2 = dram_in("scorer2", (D, 1))          # pre-normalized
    iota_gw = dram_in("iota_gw", (128, GW))       # row = 0..GW-1, all partitions
    iota_col = dram_in("iota_col", (128, 1))      # p * C_SC
    identity = dram_in("identity", (128, 128))
    negpad = dram_in("negpad", (1, 128))          # -1e30 row
    idx_d = dram_in("idx", (T, 16, cfg.ncol8), I16)
    dstloc_d = dram_in("dstloc", (T, 128, cfg.ncol), U8)
    w_d = dram_in("wv", (T, 128, cfg.ncol), I16)

    geom = _geom(cfg)
    NBLK = sum(len(hh) for _, hh in geom)
    # int8 output, transposed [D, NPART] + per-(feature, row-block) amax scales
    out_d = nc.dram_tensor("out", [T, D, NPART], I8, kind="ExternalOutput").ap()
    scales_d = nc.dram_tensor("scales", [T, D, NBLK], F32,
                              kind="ExternalOutput").ap()

    with tile.TileContext(nc) as tc:
        import contextlib
        ctx = contextlib.ExitStack()
        with ctx:
            sb = ctx.enter_context(tc.tile_pool(name="sb", bufs=1))
            meta = ctx.enter_context(tc.tile_pool(name="meta", bufs=1))
            rcp = ctx.enter_context(tc.tile_pool(name="rcp", bufs=1))
            xgp = ctx.enter_context(tc.tile_pool(name="xgp", bufs=3))
            stp = ctx.enter_context(tc.tile_pool(name="stp", bufs=8))
            gtp = ctx.enter_context(tc.tile_pool(name="gtp", bufs=3))
            drp = ctx.enter_context(tc.tile_pool(name="drp", bufs=4))
            psg = ctx.enter_context(tc.tile_pool(name="psg", bufs=2, space="PSUM"))
            pso = ctx.enter_context(tc.tile_pool(name="pso", bufs=1, space="PSUM"))
            tkp = ctx.enter_context(tc.tile_pool(name="tkp", bufs=1))
            dram = ctx.enter_context(tc.tile_pool(name="dram", bufs=1, space="DRAM"))

            # constants
            iota_sb = sb.tile([128, GW], F32, tag="iota")
            nc.sync.dma_start(out=iota_sb[:], in_=iota_gw[:])
            ident_sb = sb.tile([128, 128], F32, tag="ident")
            nc.sync.dma_start(out=ident_sb[:], in_=identity[:])
            iotac_sb = sb.tile([128, 1], F32, tag="iotac")
            nc.sync.dma_start(out=iotac_sb[:], in_=iota_col[:])
            neg_sb = sb.tile([1, 128], F32, tag="negp")
            nc.sync.dma_start(out=neg_sb[:], in_=negpad[:])
            sc2_sb = sb.tile([128, 1], F32, tag="sc2")
            nc.sync.dma_start(out=sc2_sb[:], in_=scorer2[:])
            gW_sb, gU_sb, gb_sb = [], [], []
            for i in range(3):
                a = sb.tile([128, 128], F32, name=f"gw{i}", tag=f"gw{i}")
                nc.sync.dma_start(out=a[:], in_=gWT2[i])
                gW_sb.append(a)
                b = sb.tile([128, 128], F32, name=f"gu{i}", tag=f"gu{i}")
                nc.sync.dma_start(out=b[:], in_=gUT2[i])
                gU_sb.append(b)
                c = sb.tile([128, 128], F32, name=f"gb{i}", tag=f"gb{i}")
                nc.sync.dma_start(out=c[:], in_=gb2[i])
                gb_sb.append(c)
            qn1_sb = []
            for t in range(T):
                q = sb.tile([128, 128], F32, name=f"qn1_{t}", tag=f"qn1_{t}")
                nc.sync.dma_start(out=q[:], in_=qn1[t])
                qn1_sb.append(q)

            # persistent DRAM buffers
            nodes_sl = [dram.tile([NPART, D], F32, name=f"nsl{t}", tag=f"nsl{t}")
                        for t in range(T)]
            nodes_full = [dram.tile([N, D], F32, name=f"nfl{t}", tag=f"nfl{t}",
                                    addr_space="Shared") for t in range(T)]
            nodes_loc = [dram.tile([N, D], F32, name=f"nlc{t}", tag=f"nlc{t}")
                         for t in range(T)]
            h_slice = [dram.tile([NPART, D], F16, name=f"hsl{t}", tag=f"hsl{t}")
                       for t in range(T)]
            h_full = [dram.tile([N, D], F16, name=f"hfl{t}", tag=f"hfl{t}",
                                addr_space="Shared") for t in range(T)]
            sc_slice = [dram.tile([1, NPART], F32, name=f"ssl{t}", tag=f"ssl{t}")
                        for t in range(T)]
            h_loc = [dram.tile([N, D], F16, name=f"hlc{t}", tag=f"hlc{t}")
                     for t in range(T)]
            sc_full = [dram.tile([1, PADN], F32, name=f"sfl{t}", tag=f"sfl{t}",
                                 addr_space="Shared") for t in range(T)]
            cand_dram = dram.tile([1, NCAND], F32, tag="cand", bufs=2)
            sorted_dram = dram.tile([129, 2], F32, tag="sorted", bufs=2)

            qn2_sb = [sb.tile([128, 128], F32, name=f"qn2_{t}", tag=f"qn2_{t}")
                      for t in range(T)]
            qn2h_sb = [sb.tile([128, 128], F16, name=f"qn2h_{t}", tag=f"qn2h_{t}")
                       for t in range(T)]

            # reconstruct f32 nodes shard from 24-bit planes, then AllGather:
            # z = (hi + lo/256 - 0.5) / 1024
            FLAT = NPART * D // 128          # flat columns per t (partition-major)
            NCH = 5
            CH = FLAT // NCH
            assert CH * NCH == FLAT
            for t in range(T):
                hi_flat = nhi_d[t].rearrange("a d -> (a d)").rearrange(
                    "(p c) -> p c", c=FLAT)
                lo_flat = nlo_d[t].rearrange("a d -> (a d)").rearrange(
                    "(p c) -> p c", c=FLAT)
                sl_flat = nodes_sl[t][:].rearrange("a d -> (a d)").rearrange(
                    "(p c) -> p c", c=FLAT)
                for k in range(NCH):
                    cs = slice(k * CH, (k + 1) * CH)
                    rhi = rcp.tile([128, CH], I16, tag="rhi")
                    nc.sync.dma_start(out=rhi[:], in_=hi_flat[:, cs])
                    rlo = rcp.tile([128, CH], U8, tag="rlo")
                    nc.sync.dma_start(out=rlo[:], in_=lo_flat[:, cs])
                    rhf = rcp.tile([128, CH], F32, tag="rhf")
                    nc.vector.tensor_copy(out=rhf[:], in_=rhi[:])
                    rlf = rcp.tile([128, CH], F32, tag="rlf")
                    nc.vector.tensor_copy(out=rlf[:], in_=rlo[:])
                    rt1 = rcp.tile([128, CH], F32, tag="rt1")
                    nc.vector.tensor_scalar(out=rt1[:], in0=rlf[:],
                                            scalar1=float(1.0 / 256.0),
                                            scalar2=-0.5,
                                            op0=ALU.mult, op1=ALU.add)
                    rt2 = rcp.tile([128, CH], F32, tag="rt2")
                    nc.vector.tensor_tensor(out=rt2[:], in0=rhf[:], in1=rt1[:],
                                            op=ALU.add)
                    rz = rcp.tile([128, CH], F32, tag="rz")
                    nc.vector.tensor_scalar(out=rz[:], in0=rt2[:],
                                            scalar1=float(1.0 / NSCALE),
                                            scalar2=None, op0=ALU.mult)
                    nc.sync.dma_start(out=sl_flat[:, cs], in_=rz[:])
                nc.gpsimd.collective_compute(
                    "AllGather", ALU.bypass,
                    replica_groups=[core_ids],
                    ins=[nodes_sl[t][:].opt()],
                    outs=[nodes_full[t][:].opt()])
                nc.sync.dma_start(out=nodes_loc[t][:], in_=nodes_full[t][:])

            def spmm_pass(t, z_src_ap, qn_tile, layer):
                """One (layer, t) SpMM pass. z_src_ap: [N, D] DRAM AP
                (f32 for layer 1, fp16 for layer 2)."""
                zdt = F32 if layer == 1 else F16
                idx_sb = meta.tile([128, cfg.ncol8], I16, tag="idx")
                for s in range(8):
                    nc.sync.dma_start(out=idx_sb[16 * s:16 * (s + 1), :],
                                      in_=idx_d[t])
                dl8_sb = meta.tile([128, cfg.ncol], U8, tag="dl8")
                nc.sync.dma_start(out=dl8_sb[:], in_=dstloc_d[t])
                dl_sb = meta.tile([128, cfg.ncol], F32, tag="dl")
                nc.vector.tensor_copy(out=dl_sb[:], in_=dl8_sb[:])
                wq_sb = meta.tile([128, cfg.ncol], I16, tag="wq")
                nc.sync.dma_start(out=wq_sb[:], in_=w_d[t])
                w_sb = meta.tile([128, cfg.ncol], F32, tag="wv")
                nc.vector.tensor_copy(out=w_sb[:], in_=wq_sb[:])
                z_lo = z_src_ap[0:SPLIT, :]
                z_hi = z_src_ap[SPLIT:N, :]
                if layer == 2:
                    sc8 = gtp.tile([128, NBLK], F32, tag="sc8")
                bi = 0
                for g in range(NG):
                    r0, hh = geom[g]
                    xg = []
                    for half, zsrc in ((0, z_lo), (1, z_hi)):
                        xt = xgp.tile([128, F * 128], zdt, tag="xg",
                                      name=f"xg{layer}_{t}_{g}_{half}")
                        c0 = (g * 2 + half) * F * 8
                        # single_packet SWDGE limit: <=64 desc/engine -> 1024 idxs
                        for s0 in range(0, F, 8):
                            ns = min(8, F - s0)
                            nc.gpsimd.dma_gather(
                                out_ap=xt[:, s0 * 128:(s0 + ns) * 128]
                                .rearrange("p (s e) -> p s e", e=128),
                                in_ap=zsrc,
                                idxs_ap=idx_sb[:, c0 + s0 * 8:c0 + (s0 + ns) * 8],
                                num_idxs=ns * 128,
                                num_idxs_reg=ns * 128,
                                elem_size=128,
                            )
                        xg.append(xt)
                    gt_ps = psg.tile([128, GW], F32, tag="gt", space="PSUM")
                    nmm = 2 * F
                    k = 0
                    for half in (0, 1):
                        for s in range(F):
                            col = (g * 2 + half) * F + s
                            st = stp.tile([128, GW], zdt, tag="st",
                                          name=f"st{layer}_{t}_{g}_{half}_{s}")
                            nc.vector.tensor_scalar(
                                out=st[:], in0=iota_sb[:],
                                scalar1=dl_sb[:, col:col + 1],
                                scalar2=w_sb[:, col:col + 1],
                                op0=ALU.is_equal, op1=ALU.mult)
                            lhs = xg[half][:, s * 128:(s + 1) * 128]
                            nc.tensor.matmul(out=gt_ps[:], lhsT=lhs, rhs=st[:],
                                             start=(k == 0), stop=(k == nmm - 1))
                            k += 1
                    # copy-out descales the int16 fixed-point edge weights
                    gt_sb = gtp.tile([128, GW], zdt, tag="gts")
                    nc.scalar.activation(out=gt_sb[:], in_=gt_ps[:], func=ACT.Copy,
                                         scale=float(1.0 / WSCALE))
                    for (rr, wdt) in hh:
                        o_ps = pso.tile([128, 128], F32, tag="ops", space="PSUM", bufs=2)
                        lhs2 = gt_sb[:, rr - r0:rr - r0 + wdt]
                        rhs2 = qn_tile[:]
                        nc.tensor.matmul(out=o_ps[:wdt, :], lhsT=lhs2, rhs=rhs2,
                                         start=True, stop=True)
                        sx = drp.tile([128, 128], F32, tag="sx")
                        nc.scalar.activation(out=sx[:wdt, :], in_=o_ps[:wdt, :],
                                             func=ACT.Copy, scale=SLOPE)
                        hb = drp.tile([128, 128], F32, tag="hb")
                        nc.vector.tensor_tensor(out=hb[:wdt, :], in0=o_ps[:wdt, :],
                                                in1=sx[:wdt, :], op=ALU.max)
                        # both layers transpose h (layer 1: scores; layer 2:
                        # per-feature int8 quantization on partitions)
                        ht_ps = pso.tile([128, 128], F32, tag="htp",
                                         space="PSUM")
                        nc.tensor.transpose(out=ht_ps[:, :wdt], in_=hb[:wdt, :],
                                            identity=ident_sb[:wdt, :wdt])
                        ht_sb = drp.tile([128, 128], F32, tag="hts")
                        nc.scalar.activation(out=ht_sb[:, :wdt],
                                             in_=ht_ps[:, :wdt], func=ACT.Copy)
                        if layer == 1:
                            hb16 = drp.tile([128, 128], F16, tag="hb16")
                            nc.vector.tensor_copy(out=hb16[:wdt, :],
                                                  in_=hb[:wdt, :])
                            nc.sync.dma_start(out=h_slice[t][rr:rr + wdt, :],
                                              in_=hb16[:wdt, :])
                            s_ps = pso.tile([1, 128], F32, tag="sps", space="PSUM")
                            nc.tensor.matmul(out=s_ps[:, :wdt], lhsT=sc2_sb[:],
                                             rhs=ht_sb[:, :wdt], start=True,
                                             stop=True)
                            s_sb = drp.tile([1, 128], F32, tag="ssb")
                            nc.scalar.activation(out=s_sb[:, :wdt],
                                                 in_=s_ps[:, :wdt], func=ACT.Copy)
                            nc.sync.dma_start(
                                out=sc_slice[t][:, rr:rr + wdt],
                                in_=s_sb[:1, :wdt])
                        else:
                            # int8 quantize per feature row of ht
                            mx = drp.tile([128, 1], F32, tag="qmx")
                            nc.vector.tensor_reduce(
                                out=mx[:], in_=ht_sb[:, :wdt],
                                axis=mybir.AxisListType.X, op=ALU.max)
                            mn = drp.tile([128, 1], F32, tag="qmn")
                            nc.vector.tensor_reduce(
                                out=mn[:], in_=ht_sb[:, :wdt],
                                axis=mybir.AxisListType.X, op=ALU.min)
                            nmn = drp.tile([128, 1], F32, tag="qnm")
                            nc.vector.tensor_scalar(out=nmn[:], in0=mn[:],
                                                    scalar1=-1.0, scalar2=None,
                                                    op0=ALU.mult)
                            am = drp.tile([128, 1], F32, tag="qam")
                            nc.vector.tensor_tensor(out=am[:], in0=mx[:],
                                                    in1=nmn[:], op=ALU.max)
                            amc = drp.tile([128, 1], F32, tag="qac")
                            nc.vector.tensor_scalar(out=amc[:], in0=am[:],
                                                    scalar1=1e-30, scalar2=None,
                                                    op0=ALU.max)
                            rc = drp.tile([128, 1], F32, tag="qrc")
                            nc.vector.reciprocal(out=rc[:], in_=amc[:])
                            inv = drp.tile([128, 1], F32, tag="qin")
                            nc.vector.tensor_scalar(out=inv[:], in0=rc[:],
                                                    scalar1=127.0, scalar2=None,
                                                    op0=ALU.mult)
                            q8 = drp.tile([128, 128], I8, tag="q8")
                            nc.vector.tensor_scalar(out=q8[:, :wdt],
                                                    in0=ht_sb[:, :wdt],
                                                    scalar1=inv[:, 0:1],
                                                    scalar2=None, op0=ALU.mult)
                            nc.vector.tensor_copy(out=sc8[:, bi:bi + 1],
                                                  in_=amc[:])
                            nc.sync.dma_start(out=out_d[t, :, rr:rr + wdt],
                                              in_=q8[:, :wdt])
                        bi += 1
                if layer == 2:
                    nc.sync.dma_start(out=scales_d[t], in_=sc8[:])
                if layer == 1:
                    nc.gpsimd.collective_compute(
                        "AllGather", ALU.bypass,
                        replica_groups=[core_ids],
                        ins=[h_slice[t][:].opt()],
                        outs=[h_full[t][:].opt()])
                    nc.sync.dma_start(out=h_loc[t][:], in_=h_full[t][:])
                    nc.gpsimd.collective_compute(
                        "AllGather", ALU.bypass,
                        replica_groups=[core_ids],
                        ins=[sc_slice[t][:].opt()],
                        outs=[sc_full[t][:, 0:N].opt()])

            def topk_gru(t, q_prev):
                """Exact top-128 of sc_full[t] + matrix GRU -> qn2_sb[t]."""
                S = tkp.tile([128, C_SC], F32, tag="S")
                nc.sync.dma_start(out=S[:],
                                  in_=sc_full[t][:].rearrange("o (p c) -> (o p) c",
                                                              c=C_SC))
                if PADN > N:
                    p_t, c_t = N // C_SC, N % C_SC
                    nc.sync.dma_start(out=S[p_t:p_t + 1, c_t:C_SC],
                                      in_=negpad[0:1, 0:C_SC - c_t])
                    if p_t + 1 < 128:
                        nc.sync.dma_start(
                            out=S[p_t + 1:128, :],
                            in_=negpad[0:1, 0:1].to_broadcast(
                                [127 - p_t, C_SC]))
                vals = tkp.tile([128, 8 * R], F32, tag="vals")
                cols = tkp.tile([128, 8 * R], F32, tag="cols")
                Swork = S
                for r in range(R):
                    mx = tkp.tile([128, 8], F32, tag="mx")
                    nc.vector.max(out=mx[:], in_=Swork[:])
                    ix = tkp.tile([128, 8], mybir.dt.uint32, tag="ix")
                    nc.vector.max_index(out=ix[:], in_max=mx[:], in_values=Swork[:])
                    nc.vector.tensor_copy(out=vals[:, r * 8:(r + 1) * 8], in_=mx[:])
                    nc.vector.tensor_copy(out=cols[:, r * 8:(r + 1) * 8], in_=ix[:])
                    if r < R - 1:
                        S2 = tkp.tile([128, C_SC], F32, tag=f"Sw{r % 2}")
                        nc.vector.match_replace(out=S2[:], in_to_replace=mx[:],
                                                in_values=Swork[:],
                                                imm_value=-1e30)
                        Swork = S2
                # global node id n = p*C_SC + col
                nid = tkp.tile([128, 8 * R], F32, tag="nid")
                nc.vector.tensor_scalar(out=nid[:], in0=cols[:],
                                        scalar1=iotac_sb[:, :1], scalar2=None,
                                        op0=ALU.add)
                # broadcast all candidates to all partitions via DRAM bounce
                nc.sync.dma_start(out=cand_dram[:], in_=vals[:])
                cb = tkp.tile([128, NCAND], F32, tag="cb")
                nc.sync.dma_start(out=cb[:],
                                  in_=cand_dram[:].to_broadcast([128, NCAND]))
                rank = tkp.tile([128, 8 * R], F32, tag="rank")
                for j in range(8 * R):
                    cmp = tkp.tile([128, NCAND], F32, tag="cmp")
                    nc.vector.tensor_scalar(out=cmp[:], in0=cb[:],
                                            scalar1=vals[:, j:j + 1], scalar2=None,
                                            op0=ALU.is_gt)
                    nc.vector.tensor_reduce(out=rank[:, j:j + 1], in_=cmp[:],
                                            axis=mybir.AxisListType.X, op=ALU.add)
                nc.vector.tensor_scalar(out=rank[:], in0=rank[:], scalar1=128.0,
                                        scalar2=None, op0=ALU.min)
                ri = tkp.tile([128, 8 * R], I32, tag="ri")
                nc.vector.tensor_copy(out=ri[:], in_=rank[:])
                pairs = tkp.tile([128, 16 * R], F32, tag="pairs")
                nc.vector.tensor_copy(
                    out=pairs[:].rearrange("p (j two) -> p j two", two=2)[:, :, 0],
                    in_=nid[:])
                nc.vector.tensor_copy(
                    out=pairs[:].rearrange("p (j two) -> p j two", two=2)[:, :, 1],
                    in_=vals[:])
                for j in range(8 * R):
                    nc.gpsimd.indirect_dma_start(
                        out=sorted_dram[:],
                        out_offset=bass.IndirectOffsetOnAxis(
                            ap=ri[:, j:j + 1], axis=0),
                        in_=pairs[:, 2 * j:2 * j + 2],
                        in_offset=None)
                sv = tkp.tile([128, 2], F32, tag="sv")
                nc.sync.dma_start(out=sv[:], in_=sorted_dram[0:128, :])
                nidx = tkp.tile([128, 1], I32, tag="nidx")
                nc.vector.tensor_copy(out=nidx[:], in_=sv[:, 0:1])
                tanhv = tkp.tile([128, 1], F32, tag="tanhv")
                nc.scalar.activation(out=tanhv[:], in_=sv[:, 1:2], func=ACT.Tanh)
                zsel16 = tkp.tile([128, 128], F16, tag="zsel16")
                nc.gpsimd.indirect_dma_start(
                    out=zsel16[:], out_offset=None,
                    in_=h_full[t][:],
                    in_offset=bass.IndirectOffsetOnAxis(ap=nidx[:, :1], axis=0))
                zsel = tkp.tile([128, 128], F32, tag="zsel")
                nc.vector.tensor_copy(out=zsel[:], in_=zsel16[:])
                zs2 = tkp.tile([128, 128], F32, tag="zs2")
                nc.scalar.activation(out=zs2[:], in_=zsel[:], func=ACT.Copy,
                                     scale=tanhv[:, :1])
                zt_ps = pso.tile([128, 128], F32, tag="ztp", space="PSUM")
                nc.tensor.transpose(out=zt_ps[:], in_=zs2[:], identity=ident_sb[:])
                ztop = tkp.tile([128, 128], F32, tag="ztop")
                nc.scalar.activation(out=ztop[:], in_=zt_ps[:], func=ACT.Copy)
                # matrix GRU
                gates = []
                rstq = None
                for i in range(3):
                    g_ps = pso.tile([128, 128], F32, tag="gps", space="PSUM")
                    nc.tensor.matmul(out=g_ps[:], lhsT=gW_sb[i][:], rhs=ztop[:],
                                     start=True, stop=False)
                    other = q_prev if i < 2 else rstq
                    nc.tensor.matmul(out=g_ps[:], lhsT=gU_sb[i][:], rhs=other[:],
                                     start=False, stop=True)
                    gsum = tkp.tile([128, 128], F32, tag=f"gsum{i}")
                    nc.vector.tensor_tensor(out=gsum[:], in0=g_ps[:],
                                            in1=gb_sb[i][:], op=ALU.add)
                    gact = tkp.tile([128, 128], F32, tag=f"gact{i}")
                    nc.scalar.activation(out=gact[:], in_=gsum[:],
                                         func=(ACT.Sigmoid if i < 2 else ACT.Tanh))
                    gates.append(gact)
                    if i == 1:
                        rstq = tkp.tile([128, 128], F32, tag="rstq")
                        nc.vector.tensor_tensor(out=rstq[:], in0=gates[1][:],
                                                in1=q_prev[:], op=ALU.mult)
                upd, h_cap = gates[0], gates[2]
                dql = tkp.tile([128, 128], F32, tag="dql")
                nc.vector.tensor_tensor(out=dql[:], in0=h_cap[:], in1=q_prev[:],
                                        op=ALU.subtract)
                udl = tkp.tile([128, 128], F32, tag="udl")
                nc.vector.tensor_tensor(out=udl[:], in0=upd[:], in1=dql[:],
                                        op=ALU.mult)
                nc.vector.tensor_tensor(out=qn2_sb[t][:], in0=q_prev[:],
                                        in1=udl[:], op=ALU.add)
                nc.vector.tensor_copy(out=qn2h_sb[t][:], in_=qn2_sb[t][:])
                return qn2_sb[t]

            # ---- program ----
            bisect = os.environ.get("KBISECT", "")
            if bisect.startswith("spmm"):
                npass = int(bisect[4:] or 2 * T)
                for i in range(npass):
                    spmm_pass(i % T, nodes_loc[i % T][:], qn1_sb[i % T], layer=1)
            else:
                for t in range(T):
                    spmm_pass(t, nodes_loc[t][:], qn1_sb[t], layer=1)
                qprev = sb.tile([128, 128], F32, name="winit2_sb", tag="winit2")
                nc.sync.dma_start(out=qprev[:], in_=winit2[:])
                for t in range(T):
                    qprev = topk_gru(t, qprev)
                for t in range(T):
                    spmm_pass(t, h_loc[t][:], qn2h_sb[t], layer=2)

    nc.compile()
    # memoize the BIR serialization: the module is immutable after compile,
    # but run_bass_via_pjrt re-lowers (and re-serializes ~77MB of BIR JSON)
    # on every invocation
    _tjb_cache = {}
    _orig_tjb = nc.to_json_bytes

    def _cached_tjb():
        if "b" not in _tjb_cache:
            _tjb_cache["b"] = _orig_tjb()
        return _tjb_cache["b"]

    nc.to_json_bytes = _cached_tjb
    return nc


# ---------------------------------------------------------------- entry point
_CACHE = {}
_LAST_IN_MAPS = None
_LAST_CFG = None

# full-problem constants (hardcoded per contract)
_T, _N, _E, _NCORES = 6, 50000, 1600000, 8


def _pack_nodes_24bit(zs):
    """[.., ] f32 -> (int16 hi, uint8 lo): z ~ (hi + lo/256 - 0.5)/1024."""
    s = zs.astype(np.float64) * NSCALE
    hi = np.round(s)
    assert np.abs(hi).max() < 32767, "nodes exceed 24-bit fixed-point range"
    lo = np.clip(np.round((s - hi + 0.5) * 256.0), 0, 255)
    return hi.astype(np.int16), lo.astype(np.uint8)


def assemble_out(res, cfg=None):
    """Dequantize per-core int8 outputs -> full [T, N, D] f32."""
    cfg = cfg or _LAST_CFG
    geom = _geom(cfg)
    bi_of_row = np.zeros(cfg.NPART, np.int64)
    bi = 0
    for _, hh in geom:
        for (rr, wdt) in hh:
            bi_of_row[rr:rr + wdt] = bi
            bi += 1
    outs = []
    for c in range(cfg.NCORES):
        q = res.results[c]["out"].astype(np.float32)    # [T, D, NPART]
        am = res.results[c]["scales"]                   # [T, D, NBLK]
        amr = am[:, :, bi_of_row]                       # [T, D, NPART]
        outs.append(np.transpose(q * (amr * (1.0 / 127.0)), (0, 2, 1)))
    return np.concatenate(outs, axis=1).astype(np.float32)


def kernel(nodes, edge_src, edge_dst, edge_weight,
           W_init1, scorer1, gate_W1, gate_U1, gate_b1,
           W_init2, scorer2, gate_W2, gate_U2, gate_b2):
    nodes = np.ascontiguousarray(np.asarray(nodes, np.float32))
    T, N, D = nodes.shape
    E = np.asarray(edge_src).shape[1]
    gw = int(os.environ.get("KGW", "256"))
    cfg = Cfg(T, N, E, _NCORES, gw=gw, topk_rounds=2)
    idx, dstloc, wpack = _pack_edges(
        cfg, np.asarray(edge_src), np.asarray(edge_dst),
        np.asarray(edge_weight, np.float32))
    qn1 = _host_gru_layer1(cfg, nodes, np.asarray(W_init1, np.float32),
                           np.asarray(scorer1, np.float32),
                           np.asarray(gate_W1, np.float32),
                           np.asarray(gate_U1, np.float32),
                           np.asarray(gate_b1, np.float32))
    key = (T, N, E, cfg.F_GH, cfg.GW, cfg.R)
    if key not in _CACHE:
        _CACHE[key] = _build(cfg)
    nc = _CACHE[key]

    sc2n = (np.asarray(scorer2, np.float32)
            / np.float32(np.linalg.norm(scorer2))).astype(np.float32)
    shared = {
        "qn1": qn1,
        "gWT2": np.ascontiguousarray(
            np.transpose(np.asarray(gate_W2, np.float32), (0, 2, 1))),
        "gUT2": np.ascontiguousarray(
            np.transpose(np.asarray(gate_U2, np.float32), (0, 2, 1))),
        "gb2": np.asarray(gate_b2, np.float32),
        "winit2": np.asarray(W_init2, np.float32),
        "scorer2": sc2n,
        "iota_gw": np.tile(np.arange(cfg.GW, dtype=np.float32), (128, 1)),
        "iota_col": (np.arange(128, dtype=np.float32) * cfg.C_SC)[:, None],
        "identity": np.eye(128, dtype=np.float32),
        "negpad": np.full((1, 128), -1e30, np.float32),
    }
    in_maps = []
    for c in range(cfg.NCORES):
        m = dict(shared)
        nhi, nlo = _pack_nodes_24bit(
            nodes[:, c * cfg.NPART:(c + 1) * cfg.NPART, :])
        m["nhi"] = nhi
        m["nlo"] = nlo
        m["idx"] = idx[c]
        m["dstloc"] = dstloc[c]
        m["wv"] = wpack[c]
        in_maps.append(m)
    global _LAST_IN_MAPS, _LAST_CFG
    _LAST_IN_MAPS = in_maps
    _LAST_CFG = cfg
    res = run_bass_kernel_spmd(nc, in_maps, list(range(cfg.NCORES)))
    return assemble_out(res, cfg)


# revision 21
# speedup vs baseline: 4.4033x; 1.0818x over previous
"""EvolveGCN (EGCN-H, 2 GRCU layers) Trainium2 Bass kernel, 8-way SPMD.

Strategy (dst-sharded graph parallel, transfer-optimized):
- 8 cores each own a contiguous range of N/8 destination nodes. Edges are
  routed (host-side) to their dst-owner core, grouped by 256-wide dst groups
  and by src half (int16 gather-index limit), padded to a fixed per-group
  subchunk count so the device program is static and identical on all cores.
- Host->device traffic is minimized (the axon tunnel is ~15-60MB/s and
  dominates wall time; device exec is ~0.15s):
  nodes ship SHARDED (N/8 rows per core) as 24-bit fixed-point planes
  (int16 hi + uint8 lo, reconstructed on-device: z=(hi+lo/256-0.5)/1024)
  and are AllGathered on-device over NeuronLink; gather indices ship as
  the 16-row wrapped band (replicated to 128 partitions on-device);
  dst-locations ship uint8; edge weights ship int16 fixed-point (w*32768,
  descaled for free via the PSUM->SBUF copy scale); h ships/stores fp16;
  the output ships int8 with per-(feature, 128-row-block) amax scales,
  dequantized host-side in assemble_out().
- Precision split (validated on the exact instance): layer-1 SpMM stays f32
  because the layer-2 top-k selection scores derive from h and the rank-128
  score gap is ~5e-3 - any fp16 rounding upstream of the scores flips the
  selected set and cascades through the sequential matrix GRU (rel err 0.3+).
  Layer-2's data path (h storage, gathers, S_T, both matmuls, output) is
  fp16/int8: selection happens before rounding, everything after is smooth.
  End-to-end sim (and measured device) rel err: 5.1e-3 vs the 2e-2 gate.
- segment_sum linearity: segsum(w * (Z@Q)[src], dst) == segsum(w*Z[src], dst) @ Q.
  Per 128-edge subchunk the core dma_gathers 128 rows of Z, builds the
  weighted one-hot S_T[e, d] = w_e * (dst_e == d) with one fused DVE
  tensor_scalar (is_equal x mult against a constant iota), and accumulates
  G.T = X.T @ S_T in PSUM with one matmul. After a group finishes:
  out = lrelu(G @ Q) via one more matmul.
- Layer boundary: per-step AllGather of the h slices (+ device-computed
  layer-2 scores); on-device exact top-k (vector.max8/max_index + global
  rank by count + indirect rank-scatter) and the 128x128 matrix GRU produce
  layer-2's evolved weights. Layer-1's evolved weights are host-precomputed
  (tiny sequential GRU on pure inputs, replicated - see sharding hint).
"""
import os
import sys

for _p in ("/opt/trn_rl_repo", "/root/.axon_site/_ro/trn_rl_repo"):
    if os.path.isdir(_p) and _p not in sys.path:
        sys.path.insert(0, _p)

import numpy as np

# Persistent jax compilation cache: run_bass_kernel_spmd re-jits a fresh
# closure per call, so without this every invocation re-runs the full
# walrus BIR->NEFF compile (~10-40s). The cache is keyed on the HLO hash;
# the executable still loads + runs on-device per call.
import jax as _jax

for _k, _v in (("jax_compilation_cache_dir", "/tmp/jax_cc_cache"),
               ("jax_persistent_cache_min_compile_time_secs", 0.0),
               ("jax_persistent_cache_min_entry_size_bytes", 0)):
    try:
        _jax.config.update(_k, _v)
    except Exception:
        pass

import concourse.bass as bass
import concourse.bacc as bacc
import concourse.mybir as mybir
import concourse.tile as tile
from concourse.bass_utils import run_bass_kernel_spmd

F32 = mybir.dt.float32
F16 = mybir.dt.float16
I16 = mybir.dt.int16
I32 = mybir.dt.int32
I8 = mybir.dt.int8
U8 = mybir.dt.uint8
ALU = mybir.AluOpType
ACT = mybir.ActivationFunctionType
SLOPE = float((1.0 / 8.0 + 1.0 / 3.0) / 2.0)  # rrelu eval-mode slope
WSCALE = 32768.0  # edge-weight int16 fixed-point scale
NSCALE = 1024.0   # nodes 24-bit fixed-point scale (range +-32, frac 2^-18)


class Cfg:
    def __init__(self, T, N, E, ncores, gw=256, topk_rounds=2):
        self.T, self.N, self.E, self.NCORES = T, N, E, ncores
        assert N % ncores == 0
        self.NPART = N // ncores          # dst nodes per core
        self.GW = gw                      # dst group width (matmul free dim)
        self.NG = -(-self.NPART // gw)    # groups per core
        self.SPLIT = min(32768 - (32768 % 128), -(-N // 2 // 128) * 128)
        if N <= 32767:
            self.SPLIT = -(-N // 2 // 128) * 128  # exercise both halves
        self.SPLIT = min(self.SPLIT, N)
        self.D = 128
        self.K = 128
        self.C_SC = -(-N // 128)          # score columns per partition
        self.PADN = 128 * self.C_SC
        self.R = topk_rounds              # rounds of per-partition max8
        self.NCAND = 128 * 8 * topk_rounds
        self.F_GH = None                  # set from data
        self.ncol = None
        self.ncol8 = None

    def set_fgh(self, f):
        self.F_GH = f
        self.ncol = self.NG * 2 * f           # metadata columns per t
        self.ncol8 = self.NG * 2 * f * 8      # idx columns per t


# ---------------------------------------------------------------- host prep
def _pack_edges(cfg, edge_src, edge_dst, edge_w):
    """Per-core static streams. Returns (idx, dstloc, w) arrays:
    idx   [NCORES, T, 16, ncol8]  int16   (16-row wrapped band)
    dstloc[NCORES, T, 128, ncol]  float16 (values 0..GW-1, exact in fp16)
    wpack [NCORES, T, 128, ncol]  int16   (w * 32768 fixed-point)
    """
    T, NG, GW, NPART, SPLIT = cfg.T, cfg.NG, cfg.GW, cfg.NPART, cfg.SPLIT
    NC = cfg.NCORES
    # fixed subchunks per (group, half): global max over all (core,t,g,half)
    maxc = 0
    percore_key = []
    for t in range(T):
        dst, src = edge_dst[t], edge_src[t]
        key = ((dst // NPART) * NG + (dst % NPART) // GW) * 2 + (src >= SPLIT)
        percore_key.append(key.astype(np.int64))
        maxc = max(maxc, int(np.bincount(key, minlength=NC * NG * 2).max()))
    F = -(-maxc // 128)
    cfg.set_fgh(F)

    idx = np.zeros((NC, T, 16, cfg.ncol8), np.int16)
    dstloc = np.zeros((NC, T, 128, cfg.ncol), np.uint8)
    wpack = np.zeros((NC, T, 128, cfg.ncol), np.int16)
    for t in range(T):
        dst, src, w = edge_dst[t], edge_src[t], edge_w[t]
        key = percore_key[t]
        order = np.argsort(key, kind="stable")
        key_s = key[order]
        src_s, dst_s, w_s = src[order], dst[order], w[order]
        cnt = np.bincount(key_s, minlength=NC * NG * 2)
        start = np.concatenate([[0], np.cumsum(cnt)[:-1]])
        pos = np.arange(len(key_s)) - start[key_s]  # position within block
        core = key_s // (NG * 2)
        blk = key_s % (NG * 2)                      # (g*2+half) within core
        i = pos                                     # stream slot in block
        s_sub, p_row = i // 128, i % 128
        col = blk * F + s_sub
        dl = (dst_s % NPART) % GW
        half = blk % 2
        iv = src_s - half * SPLIT
        assert iv.max() < 32768 and iv.min() >= 0
        dstloc[core, t, p_row, col] = dl.astype(np.uint8)
        wpack[core, t, p_row, col] = np.minimum(
            np.round(w_s.astype(np.float64) * WSCALE), WSCALE - 1).astype(np.int16)
        # 16-row wrapped indices
        r = i % 16
        j = blk * F * 8 + i // 16
        idx[core, t, r, j] = iv.astype(np.int16)
    return idx, dstloc, wpack


def _host_gru_layer1(cfg, nodes, W_init, scorer, gW, gU, gb):
    """Exact fp32 replica of the reference layer-1 weight evolution."""
    sn = np.float32(np.linalg.norm(scorer))
    Q = W_init.copy()
    qns = []
    for t in range(cfg.T):
        Z = nodes[t]
        scores = (Z @ scorer)[:, 0] / sn
        idx = np.argsort(-scores, kind="stable")[: cfg.K]
        z_topk = (Z[idx] * np.tanh(scores[idx])[:, None]).T
        upd = 1.0 / (1.0 + np.exp(-(gW[0] @ z_topk + gU[0] @ Q + gb[0])))
        rst = 1.0 / (1.0 + np.exp(-(gW[1] @ z_topk + gU[1] @ Q + gb[1])))
        h_cap = np.tanh(gW[2] @ z_topk + gU[2] @ (rst * Q) + gb[2])
        Q = (1.0 - upd) * Q + upd * h_cap
        qns.append(Q.copy())
    return np.stack(qns).astype(np.float32)


def _geom(cfg):
    """Group geometry: list of (r0, [(row0, width<=128), ...]) per group."""
    geom = []
    for g in range(cfg.NG):
        r0 = g * cfg.GW
        r1 = min(r0 + cfg.GW, cfg.NPART)
        hh = []
        x = r0
        while x < r1:
            wdt = min(128, r1 - x)
            hh.append((x, wdt))
            x += wdt
        geom.append((r0, hh))
    return geom


# ---------------------------------------------------------------- device build
def _build(cfg):
    nc = bacc.Bacc("TRN2", target_bir_lowering=False, debug=False,
                   num_devices=cfg.NCORES)
    T, N, D, GW, NG, F, NPART = cfg.T, cfg.N, cfg.D, cfg.GW, cfg.NG, cfg.F_GH, cfg.NPART
    SPLIT, C_SC, PADN, R = cfg.SPLIT, cfg.C_SC, cfg.PADN, cfg.R
    NCAND = cfg.NCAND
    core_ids = list(range(cfg.NCORES))

    def dram_in(name, shape, dtype=F32):
        return nc.dram_tensor(name, list(shape), dtype, kind="ExternalInput").ap()

    nhi_d = dram_in("nhi", (T, NPART, D), I16)   # round(z*1024)
    nlo_d = dram_in("nlo", (T, NPART, D), U8)    # frac plane: (resid+.5)*256
    qn1 = dram_in("qn1", (T, D, D))
    gWT2 = dram_in("gWT2", (3, D, D))
    gUT2 = dram_in("gUT2", (3, D, D))
    gb2 = dram_in("gb2", (3, D, D))
    winit2 = dram_in("winit2", (D, D))
    scorer2 = dram_in("scorer2", (D, 1))          # pre-normalized
    iota_gw = dram_in("iota_gw", (128, GW))       # row = 0..GW-1, all partitions
    iota_col = dram_in("iota_col", (128, 1))      # p * C_SC
    identity = dram_in("identity", (128, 128))
    negpad = dram_in("negpad", (1, 128))          # -1e30 row
    idx_d = dram_in("idx", (T, 16, cfg.ncol8), I16)
    dstloc_d = dram_in("dstloc", (T, 128, cfg.ncol), U8)
    w_d = dram_in("wv", (T, 128, cfg.ncol), I16)

    geom = _geom(cfg)
    NBLK = sum(len(hh) for _, hh in geom)
    # int8 output, transposed [D, NPART] + per-(feature, row-block) amax scales
    out_d = nc.dram_tensor("out", [T, D, NPART], I8, kind="ExternalOutput").ap()
    scales_d = nc.dram_tensor("scales", [T, D, NBLK], F32,
                              kind="ExternalOutput").ap()

    with tile.TileContext(nc) as tc:
        import contextlib
        ctx = contextlib.ExitStack()
        with ctx:
            sb = ctx.enter_context(tc.tile_pool(name="sb", bufs=1))
            meta = ctx.enter_context(tc.tile_pool(name="meta", bufs=1))
            rcp = ctx.enter_context(tc.tile_pool(name="rcp", bufs=1))
            xgp = ctx.enter_context(tc.tile_pool(name="xgp", bufs=3))
            stp = ctx.enter_context(tc.tile_pool(name="stp", bufs=8))
            gtp = ctx.enter_context(tc.tile_pool(name="gtp", bufs=3))
            drp = ctx.enter_context(tc.tile_pool(name="drp", bufs=4))
            psg = ctx.enter_context(tc.tile_pool(name="psg", bufs=2, space="PSUM"))
            pso = ctx.enter_context(tc.tile_pool(name="pso", bufs=1, space="PSUM"))
            tkp = ctx.enter_context(tc.tile_pool(name="tkp", bufs=1))
            dram = ctx.enter_context(tc.tile_pool(name="dram", bufs=1, space="DRAM"))

            # constants
            iota_sb = sb.tile([128, GW], F32, tag="iota")
            nc.sync.dma_start(out=iota_sb[:], in_=iota_gw[:])
            ident_sb = sb.tile([128, 128], F32, tag="ident")
            nc.sync.dma_start(out=ident_sb[:], in_=identity[:])
            iotac_sb = sb.tile([128, 1], F32, tag="iotac")
            nc.sync.dma_start(out=iotac_sb[:], in_=iota_col[:])
            neg_sb = sb.tile([1, 128], F32, tag="negp")
            nc.sync.dma_start(out=neg_sb[:], in_=negpad[:])
            sc2_sb = sb.tile([128, 1], F32, tag="sc2")
            nc.sync.dma_start(out=sc2_sb[:], in_=scorer2[:])
            gW_sb, gU_sb, gb_sb = [], [], []
            for i in range(3):
                a = sb.tile([128, 128], F32, name=f"gw{i}", tag=f"gw{i}")
                nc.sync.dma_start(out=a[:], in_=gWT2[i])
                gW_sb.append(a)
                b = sb.tile([128, 128], F32, name=f"gu{i}", tag=f"gu{i}")
                nc.sync.dma_start(out=b[:], in_=gUT2[i])
                gU_sb.append(b)
                c = sb.tile([128, 128], F32, name=f"gb{i}", tag=f"gb{i}")
                nc.sync.dma_start(out=c[:], in_=gb2[i])
                gb_sb.append(c)
            qn1_sb = []
            for t in range(T):
                q = sb.tile([128, 128], F32, name=f"qn1_{t}", tag=f"qn1_{t}")
                nc.sync.dma_start(out=q[:], in_=qn1[t])
                qn1_sb.append(q)

            # persistent DRAM buffers
            nodes_sl = [dram.tile([NPART, D], F32, name=f"nsl{t}", tag=f"nsl{t}")
                        for t in range(T)]
            nodes_full = [dram.tile([N, D], F32, name=f"nfl{t}", tag=f"nfl{t}",
                                    addr_space="Shared") for t in range(T)]
            nodes_loc = [dram.tile([N, D], F32, name=f"nlc{t}", tag=f"nlc{t}")
                         for t in range(T)]
            h_slice = [dram.tile([NPART, D], F16, name=f"hsl{t}", tag=f"hsl{t}")
                       for t in range(T)]
            h_full = [dram.tile([N, D], F16, name=f"hfl{t}", tag=f"hfl{t}",
                                addr_space="Shared") for t in range(T)]
            sc_slice = [dram.tile([1, NPART], F32, name=f"ssl{t}", tag=f"ssl{t}")
                        for t in range(T)]
            h_loc = [dram.tile([N, D], F16, name=f"hlc{t}", tag=f"hlc{t}")
                     for t in range(T)]
            sc_full = [dram.tile([1, PADN], F32, name=f"sfl{t}", tag=f"sfl{t}",
                                 addr_space="Shared") for t in range(T)]
            cand_dram = dram.tile([1, NCAND], F32, tag="cand", bufs=2)
            sorted_dram = dram.tile([129, 2], F32, tag="sorted", bufs=2)

            qn2_sb = [sb.tile([128, 128], F32, name=f"qn2_{t}", tag=f"qn2_{t}")
                      for t in range(T)]
            qn2h_sb = [sb.tile([128, 128], F16, name=f"qn2h_{t}", tag=f"qn2h_{t}")
                       for t in range(T)]

            # reconstruct f32 nodes shard from 24-bit planes, then AllGather:
            # z = (hi + lo/256 - 0.5) / 1024
            FLAT = NPART * D // 128          # flat columns per t (partition-major)
            NCH = 5
            CH = FLAT // NCH
            assert CH * NCH == FLAT
            for t in range(T):
                hi_flat = nhi_d[t].rearrange("a d -> (a d)").rearrange(
                    "(p c) -> p c", c=FLAT)
                lo_flat = nlo_d[t].rearrange("a d -> (a d)").rearrange(
                    "(p c) -> p c", c=FLAT)
                sl_flat = nodes_sl[t][:].rearrange("a d -> (a d)").rearrange(
                    "(p c) -> p c", c=FLAT)
                for k in range(NCH):
                    cs = slice(k * CH, (k + 1) * CH)
                    rhi = rcp.tile([128, CH], I16, tag="rhi")
                    nc.sync.dma_start(out=rhi[:], in_=hi_flat[:, cs])
                    rlo = rcp.tile([128, CH], U8, tag="rlo")
                    nc.sync.dma_start(out=rlo[:], in_=lo_flat[:, cs])
                    rhf = rcp.tile([128, CH], F32, tag="rhf")
                    nc.vector.tensor_copy(out=rhf[:], in_=rhi[:])
                    rlf = rcp.tile([128, CH], F32, tag="rlf")
                    nc.vector.tensor_copy(out=rlf[:], in_=rlo[:])
                    rt1 = rcp.tile([128, CH], F32, tag="rt1")
                    nc.vector.tensor_scalar(out=rt1[:], in0=rlf[:],
                                            scalar1=float(1.0 / 256.0),
                                            scalar2=-0.5,
                                            op0=ALU.mult, op1=ALU.add)
                    rt2 = rcp.tile([128, CH], F32, tag="rt2")
                    nc.vector.tensor_tensor(out=rt2[:], in0=rhf[:], in1=rt1[:],
                                            op=ALU.add)
                    rz = rcp.tile([128, CH], F32, tag="rz")
                    nc.vector.tensor_scalar(out=rz[:], in0=rt2[:],
                                            scalar1=float(1.0 / NSCALE),
                                            scalar2=None, op0=ALU.mult)
                    nc.sync.dma_start(out=sl_flat[:, cs], in_=rz[:])
                nc.gpsimd.collective_compute(
                    "AllGather", ALU.bypass,
                    replica_groups=[core_ids],
                    ins=[nodes_sl[t][:].opt()],
                    outs=[nodes_full[t][:].opt()])
                nc.sync.dma_start(out=nodes_loc[t][:], in_=nodes_full[t][:])

            def spmm_pass(t, z_src_ap, qn_tile, layer):
                """One (layer, t) SpMM pass. z_src_ap: [N, D] DRAM AP
                (f32 for layer 1, fp16 for layer 2)."""
                zdt = F32 if layer == 1 else F16
                idx_sb = meta.tile([128, cfg.ncol8], I16, tag="idx")
                for s in range(8):
                    nc.sync.dma_start(out=idx_sb[16 * s:16 * (s + 1), :],
                                      in_=idx_d[t])
                dl8_sb = meta.tile([128, cfg.ncol], U8, tag="dl8")
                nc.sync.dma_start(out=dl8_sb[:], in_=dstloc_d[t])
                dl_sb = meta.tile([128, cfg.ncol], F32, tag="dl")
                nc.vector.tensor_copy(out=dl_sb[:], in_=dl8_sb[:])
                wq_sb = meta.tile([128, cfg.ncol], I16, tag="wq")
                nc.sync.dma_start(out=wq_sb[:], in_=w_d[t])
                w_sb = meta.tile([128, cfg.ncol], F32, tag="wv")
                nc.vector.tensor_copy(out=w_sb[:], in_=wq_sb[:])
                z_lo = z_src_ap[0:SPLIT, :]
                z_hi = z_src_ap[SPLIT:N, :]
                if layer == 2:
                    sc8 = gtp.tile([128, NBLK], F32, tag="sc8")
                bi = 0
                for g in range(NG):
                    r0, hh = geom[g]
                    xg = []
                    for half, zsrc in ((0, z_lo), (1, z_hi)):
                        xt = xgp.tile([128, F * 128], zdt, tag="xg",
                                      name=f"xg{layer}_{t}_{g}_{half}")
                        c0 = (g * 2 + half) * F * 8
                        # single_packet SWDGE limit: <=64 desc/engine -> 1024 idxs
                        for s0 in range(0, F, 8):
                            ns = min(8, F - s0)
                            nc.gpsimd.dma_gather(
                                out_ap=xt[:, s0 * 128:(s0 + ns) * 128]
                                .rearrange("p (s e) -> p s e", e=128),
                                in_ap=zsrc,
                                idxs_ap=idx_sb[:, c0 + s0 * 8:c0 + (s0 + ns) * 8],
                                num_idxs=ns * 128,
                                num_idxs_reg=ns * 128,
                                elem_size=128,
                            )
                        xg.append(xt)
                    gt_ps = psg.tile([128, GW], F32, tag="gt", space="PSUM")
                    nmm = 2 * F
                    k = 0
                    for half in (0, 1):
                        for s in range(F):
                            col = (g * 2 + half) * F + s
                            st = stp.tile([128, GW], zdt, tag="st",
                                          name=f"st{layer}_{t}_{g}_{half}_{s}")
                            nc.vector.tensor_scalar(
                                out=st[:], in0=iota_sb[:],
                                scalar1=dl_sb[:, col:col + 1],
                                scalar2=w_sb[:, col:col + 1],
                                op0=ALU.is_equal, op1=ALU.mult)
                            lhs = xg[half][:, s * 128:(s + 1) * 128]
                            nc.tensor.matmul(out=gt_ps[:], lhsT=lhs, rhs=st[:],
                                             start=(k == 0), stop=(k == nmm - 1))
                            k += 1
                    # copy-out descales the int16 fixed-point edge weights
                    gt_sb = gtp.tile([128, GW], zdt, tag="gts")
                    nc.scalar.activation(out=gt_sb[:], in_=gt_ps[:], func=ACT.Copy,
                                         scale=float(1.0 / WSCALE))
                    for (rr, wdt) in hh:
                        o_ps = pso.tile([128, 128], F32, tag="ops", space="PSUM", bufs=2)
                        lhs2 = gt_sb[:, rr - r0:rr - r0 + wdt]
                        rhs2 = qn_tile[:]
                        nc.tensor.matmul(out=o_ps[:wdt, :], lhsT=lhs2, rhs=rhs2,
                                         start=True, stop=True)
                        sx = drp.tile([128, 128], F32, tag="sx")
                        nc.scalar.activation(out=sx[:wdt, :], in_=o_ps[:wdt, :],
                                             func=ACT.Copy, scale=SLOPE)
                        hb = drp.tile([128, 128], F32, tag="hb")
                        nc.vector.tensor_tensor(out=hb[:wdt, :], in0=o_ps[:wdt, :],
                                                in1=sx[:wdt, :], op=ALU.max)
                        # both layers transpose h (layer 1: scores; layer 2:
                        # per-feature int8 quantization on partitions)
                        ht_ps = pso.tile([128, 128], F32, tag="htp",
                                         space="PSUM")
                        nc.tensor.transpose(out=ht_ps[:, :wdt], in_=hb[:wdt, :],
                                            identity=ident_sb[:wdt, :wdt])
                        ht_sb = drp.tile([128, 128], F32, tag="hts")
                        nc.scalar.activation(out=ht_sb[:, :wdt],
                                             in_=ht_ps[:, :wdt], func=ACT.Copy)
                        if layer == 1:
                            hb16 = drp.tile([128, 128], F16, tag="hb16")
                            nc.vector.tensor_copy(out=hb16[:wdt, :],
                                                  in_=hb[:wdt, :])
                            nc.sync.dma_start(out=h_slice[t][rr:rr + wdt, :],
                                              in_=hb16[:wdt, :])
                            s_ps = pso.tile([1, 128], F32, tag="sps", space="PSUM")
                            nc.tensor.matmul(out=s_ps[:, :wdt], lhsT=sc2_sb[:],
                                             rhs=ht_sb[:, :wdt], start=True,
                                             stop=True)
                            s_sb = drp.tile([1, 128], F32, tag="ssb")
                            nc.scalar.activation(out=s_sb[:, :wdt],
                                                 in_=s_ps[:, :wdt], func=ACT.Copy)
                            nc.sync.dma_start(
                                out=sc_slice[t][:, rr:rr + wdt],
                                in_=s_sb[:1, :wdt])
                        else:
                            # int8 quantize per feature row of ht
                            mx = drp.tile([128, 1], F32, tag="qmx")
                            nc.vector.tensor_reduce(
                                out=mx[:], in_=ht_sb[:, :wdt],
                                axis=mybir.AxisListType.X, op=ALU.max)
                            mn = drp.tile([128, 1], F32, tag="qmn")
                            nc.vector.tensor_reduce(
                                out=mn[:], in_=ht_sb[:, :wdt],
                                axis=mybir.AxisListType.X, op=ALU.min)
                            nmn = drp.tile([128, 1], F32, tag="qnm")
                            nc.vector.tensor_scalar(out=nmn[:], in0=mn[:],
                                                    scalar1=-1.0, scalar2=None,
                                                    op0=ALU.mult)
                            am = drp.tile([128, 1], F32, tag="qam")
                            nc.vector.tensor_tensor(out=am[:], in0=mx[:],
                                                    in1=nmn[:], op=ALU.max)
                            amc = drp.tile([128, 1], F32, tag="qac")
                            nc.vector.tensor_scalar(out=amc[:], in0=am[:],
                                                    scalar1=1e-30, scalar2=None,
                                                    op0=ALU.max)
                            rc = drp.tile([128, 1], F32, tag="qrc")
                            nc.vector.reciprocal(out=rc[:], in_=amc[:])
                            inv = drp.tile([128, 1], F32, tag="qin")
                            nc.vector.tensor_scalar(out=inv[:], in0=rc[:],
                                                    scalar1=127.0, scalar2=None,
                                                    op0=ALU.mult)
                            q8 = drp.tile([128, 128], I8, tag="q8")
                            nc.vector.tensor_scalar(out=q8[:, :wdt],
                                                    in0=ht_sb[:, :wdt],
                                                    scalar1=inv[:, 0:1],
                                                    scalar2=None, op0=ALU.mult)
                            nc.vector.tensor_copy(out=sc8[:, bi:bi + 1],
                                                  in_=amc[:])
                            nc.sync.dma_start(out=out_d[t, :, rr:rr + wdt],
                                              in_=q8[:, :wdt])
                        bi += 1
                if layer == 2:
                    nc.sync.dma_start(out=scales_d[t], in_=sc8[:])
                if layer == 1:
                    nc.gpsimd.collective_compute(
                        "AllGather", ALU.bypass,
                        replica_groups=[core_ids],
                        ins=[h_slice[t][:].opt()],
                        outs=[h_full[t][:].opt()])
                    nc.sync.dma_start(out=h_loc[t][:], in_=h_full[t][:])
                    nc.gpsimd.collective_compute(
                        "AllGather", ALU.bypass,
                        replica_groups=[core_ids],
                        ins=[sc_slice[t][:].opt()],
                        outs=[sc_full[t][:, 0:N].opt()])

            def topk_gru(t, q_prev):
                """Exact top-128 of sc_full[t] + matrix GRU -> qn2_sb[t]."""
                S = tkp.tile([128, C_SC], F32, tag="S")
                nc.sync.dma_start(out=S[:],
                                  in_=sc_full[t][:].rearrange("o (p c) -> (o p) c",
                                                              c=C_SC))
                if PADN > N:
                    p_t, c_t = N // C_SC, N % C_SC
                    nc.sync.dma_start(out=S[p_t:p_t + 1, c_t:C_SC],
                                      in_=negpad[0:1, 0:C_SC - c_t])
                    if p_t + 1 < 128:
                        nc.sync.dma_start(
                            out=S[p_t + 1:128, :],
                            in_=negpad[0:1, 0:1].to_broadcast(
                                [127 - p_t, C_SC]))
                vals = tkp.tile([128, 8 * R], F32, tag="vals")
                cols = tkp.tile([128, 8 * R], F32, tag="cols")
                Swork = S
                for r in range(R):
                    mx = tkp.tile([128, 8], F32, tag="mx")
                    nc.vector.max(out=mx[:], in_=Swork[:])
                    ix = tkp.tile([128, 8], mybir.dt.uint32, tag="ix")
                    nc.vector.max_index(out=ix[:], in_max=mx[:], in_values=Swork[:])
                    nc.vector.tensor_copy(out=vals[:, r * 8:(r + 1) * 8], in_=mx[:])
                    nc.vector.tensor_copy(out=cols[:, r * 8:(r + 1) * 8], in_=ix[:])
                    if r < R - 1:
                        S2 = tkp.tile([128, C_SC], F32, tag=f"Sw{r % 2}")
                        nc.vector.match_replace(out=S2[:], in_to_replace=mx[:],
                                                in_values=Swork[:],
                                                imm_value=-1e30)
                        Swork = S2
                # global node id n = p*C_SC + col
                nid = tkp.tile([128, 8 * R], F32, tag="nid")
                nc.vector.tensor_scalar(out=nid[:], in0=cols[:],
                                        scalar1=iotac_sb[:, :1], scalar2=None,
                                        op0=ALU.add)
                # broadcast all candidates to all partitions via DRAM bounce
                nc.sync.dma_start(out=cand_dram[:], in_=vals[:])
                cb = tkp.tile([128, NCAND], F32, tag="cb")
                nc.sync.dma_start(out=cb[:],
                                  in_=cand_dram[:].to_broadcast([128, NCAND]))
                rank = tkp.tile([128, 8 * R], F32, tag="rank")
                for j in range(8 * R):
                    cmp = tkp.tile([128, NCAND], F32, tag="cmp")
                    nc.vector.tensor_scalar(out=cmp[:], in0=cb[:],
                                            scalar1=vals[:, j:j + 1], scalar2=None,
                                            op0=ALU.is_gt)
                    nc.vector.tensor_reduce(out=rank[:, j:j + 1], in_=cmp[:],
                                            axis=mybir.AxisListType.X, op=ALU.add)
                nc.vector.tensor_scalar(out=rank[:], in0=rank[:], scalar1=128.0,
                                        scalar2=None, op0=ALU.min)
                ri = tkp.tile([128, 8 * R], I32, tag="ri")
                nc.vector.tensor_copy(out=ri[:], in_=rank[:])
                pairs = tkp.tile([128, 16 * R], F32, tag="pairs")
                nc.vector.tensor_copy(
                    out=pairs[:].rearrange("p (j two) -> p j two", two=2)[:, :, 0],
                    in_=nid[:])
                nc.vector.tensor_copy(
                    out=pairs[:].rearrange("p (j two) -> p j two", two=2)[:, :, 1],
                    in_=vals[:])
                for j in range(8 * R):
                    nc.gpsimd.indirect_dma_start(
                        out=sorted_dram[:],
                        out_offset=bass.IndirectOffsetOnAxis(
                            ap=ri[:, j:j + 1], axis=0),
                        in_=pairs[:, 2 * j:2 * j + 2],
                        in_offset=None)
                sv = tkp.tile([128, 2], F32, tag="sv")
                nc.sync.dma_start(out=sv[:], in_=sorted_dram[0:128, :])
                nidx = tkp.tile([128, 1], I32, tag="nidx")
                nc.vector.tensor_copy(out=nidx[:], in_=sv[:, 0:1])
                tanhv = tkp.tile([128, 1], F32, tag="tanhv")
                nc.scalar.activation(out=tanhv[:], in_=sv[:, 1:2], func=ACT.Tanh)
                zsel16 = tkp.tile([128, 128], F16, tag="zsel16")
                nc.gpsimd.indirect_dma_start(
                    out=zsel16[:], out_offset=None,
                    in_=h_full[t][:],
                    in_offset=bass.IndirectOffsetOnAxis(ap=nidx[:, :1], axis=0))
                zsel = tkp.tile([128, 128], F32, tag="zsel")
                nc.vector.tensor_copy(out=zsel[:], in_=zsel16[:])
                zs2 = tkp.tile([128, 128], F32, tag="zs2")
                nc.scalar.activation(out=zs2[:], in_=zsel[:], func=ACT.Copy,
                                     scale=tanhv[:, :1])
                zt_ps = pso.tile([128, 128], F32, tag="ztp", space="PSUM")
                nc.tensor.transpose(out=zt_ps[:], in_=zs2[:], identity=ident_sb[:])
                ztop = tkp.tile([128, 128], F32, tag="ztop")
                nc.scalar.activation(out=ztop[:], in_=zt_ps[:], func=ACT.Copy)
                # matrix GRU
                gates = []
                rstq = None
                for i in range(3):
                    g_ps = pso.tile([128, 128], F32, tag="gps", space="PSUM")
                    nc.tensor.matmul(out=g_ps[:], lhsT=gW_sb[i][:], rhs=ztop[:],
                                     start=True, stop=False)
                    other = q_prev if i < 2 else rstq
                    nc.tensor.matmul(out=g_ps[:], lhsT=gU_sb[i][:], rhs=other[:],
                                     start=False, stop=True)
                    gsum = tkp.tile([128, 128], F32, tag=f"gsum{i}")
                    nc.vector.tensor_tensor(out=gsum[:], in0=g_ps[:],
                                            in1=gb_sb[i][:], op=ALU.add)
                    gact = tkp.tile([128, 128], F32, tag=f"gact{i}")
                    nc.scalar.activation(out=gact[:], in_=gsum[:],
                                         func=(ACT.Sigmoid if i < 2 else ACT.Tanh))
                    gates.append(gact)
                    if i == 1:
                        rstq = tkp.tile([128, 128], F32, tag="rstq")
                        nc.vector.tensor_tensor(out=rstq[:], in0=gates[1][:],
                                                in1=q_prev[:], op=ALU.mult)
                upd, h_cap = gates[0], gates[2]
                dql = tkp.tile([128, 128], F32, tag="dql")
                nc.vector.tensor_tensor(out=dql[:], in0=h_cap[:], in1=q_prev[:],
                                        op=ALU.subtract)
                udl = tkp.tile([128, 128], F32, tag="udl")
                nc.vector.tensor_tensor(out=udl[:], in0=upd[:], in1=dql[:],
                                        op=ALU.mult)
                nc.vector.tensor_tensor(out=qn2_sb[t][:], in0=q_prev[:],
                                        in1=udl[:], op=ALU.add)
                nc.vector.tensor_copy(out=qn2h_sb[t][:], in_=qn2_sb[t][:])
                return qn2_sb[t]

            # ---- program ----
            bisect = os.environ.get("KBISECT", "")
            if bisect.startswith("spmm"):
                npass = int(bisect[4:] or 2 * T)
                for i in range(npass):
                    spmm_pass(i % T, nodes_loc[i % T][:], qn1_sb[i % T], layer=1)
            else:
                for t in range(T):
                    spmm_pass(t, nodes_loc[t][:], qn1_sb[t], layer=1)
                qprev = sb.tile([128, 128], F32, name="winit2_sb", tag="winit2")
                nc.sync.dma_start(out=qprev[:], in_=winit2[:])
                for t in range(T):
                    qprev = topk_gru(t, qprev)
                for t in range(T):
                    spmm_pass(t, h_loc[t][:], qn2h_sb[t], layer=2)

    nc.compile()
    # memoize the BIR serialization: the module is immutable after compile,
    # but run_bass_via_pjrt re-lowers (and re-serializes ~77MB of BIR JSON)
    # on every invocation
    _tjb_cache = {}
    _orig_tjb = nc.to_json_bytes

    def _cached_tjb():
        if "b" not in _tjb_cache:
            _tjb_cache["b"] = _orig_tjb()
        return _tjb_cache["b"]

    nc.to_json_bytes = _cached_tjb
    return nc


# ---------------------------------------------------------------- entry point
_CACHE = {}
_LAST_IN_MAPS = None
_LAST_CFG = None

# full-problem constants (hardcoded per contract)
_T, _N, _E, _NCORES = 6, 50000, 1600000, 8


def _pack_nodes_24bit(zs):
    """[.., ] f32 -> (int16 hi, uint8 lo): z ~ (hi + lo/256 - 0.5)/1024."""
    s = zs.astype(np.float64) * NSCALE
    hi = np.round(s)
    assert np.abs(hi).max() < 32767, "nodes exceed 24-bit fixed-point range"
    lo = np.clip(np.round((s - hi + 0.5) * 256.0), 0, 255)
    return hi.astype(np.int16), lo.astype(np.uint8)


def assemble_out(res, cfg=None):
    """Dequantize per-core int8 outputs -> full [T, N, D] f32."""
    cfg = cfg or _LAST_CFG
    geom = _geom(cfg)
    bi_of_row = np.zeros(cfg.NPART, np.int64)
    bi = 0
    for _, hh in geom:
        for (rr, wdt) in hh:
            bi_of_row[rr:rr + wdt] = bi
            bi += 1
    outs = []
    for c in range(cfg.NCORES):
        q = res.results[c]["out"].astype(np.float32)    # [T, D, NPART]
        am = res.results[c]["scales"]                   # [T, D, NBLK]
        amr = am[:, :, bi_of_row]                       # [T, D, NPART]
        outs.append(np.transpose(q * (amr * (1.0 / 127.0)), (0, 2, 1)))
    return np.concatenate(outs, axis=1).astype(np.float32)


def kernel(nodes, edge_src, edge_dst, edge_weight,
           W_init1, scorer1, gate_W1, gate_U1, gate_b1,
           W_init2, scorer2, gate_W2, gate_U2, gate_b2):
    nodes = np.ascontiguousarray(np.asarray(nodes, np.float32))
    T, N, D = nodes.shape
    E = np.asarray(edge_src).shape[1]
    gw = int(os.environ.get("KGW", "256"))
    cfg = Cfg(T, N, E, _NCORES, gw=gw, topk_rounds=2)
    idx, dstloc, wpack = _pack_edges(
        cfg, np.asarray(edge_src), np.asarray(edge_dst),
        np.asarray(edge_weight, np.float32))
    qn1 = _host_gru_layer1(cfg, nodes, np.asarray(W_init1, np.float32),
                           np.asarray(scorer1, np.float32),
                           np.asarray(gate_W1, np.float32),
                           np.asarray(gate_U1, np.float32),
                           np.asarray(gate_b1, np.float32))
    key = (T, N, E, cfg.F_GH, cfg.GW, cfg.R)
    if key not in _CACHE:
        _CACHE[key] = _build(cfg)
    nc = _CACHE[key]

    sc2n = (np.asarray(scorer2, np.float32)
            / np.float32(np.linalg.norm(scorer2))).astype(np.float32)
    shared = {
        "qn1": qn1,
        "gWT2": np.ascontiguousarray(
            np.transpose(np.asarray(gate_W2, np.float32), (0, 2, 1))),
        "gUT2": np.ascontiguousarray(
            np.transpose(np.asarray(gate_U2, np.float32), (0, 2, 1))),
        "gb2": np.asarray(gate_b2, np.float32),
        "winit2": np.asarray(W_init2, np.float32),
        "scorer2": sc2n,
        "iota_gw": np.tile(np.arange(cfg.GW, dtype=np.float32), (128, 1)),
        "iota_col": (np.arange(128, dtype=np.float32) * cfg.C_SC)[:, None],
        "identity": np.eye(128, dtype=np.float32),
        "negpad": np.full((1, 128), -1e30, np.float32),
    }
    in_maps = []
    for c in range(cfg.NCORES):
        m = dict(shared)
        nhi, nlo = _pack_nodes_24bit(
            nodes[:, c * cfg.NPART:(c + 1) * cfg.NPART, :])
        m["nhi"] = nhi
        m["nlo"] = nlo
        m["idx"] = idx[c]
        m["dstloc"] = dstloc[c]
        m["wv"] = wpack[c]
        in_maps.append(m)
    global _LAST_IN_MAPS, _LAST_CFG
    _LAST_IN_MAPS = in_maps
    _LAST_CFG = cfg
    res = run_bass_kernel_spmd(nc, in_maps, list(range(cfg.NCORES)))
    return assemble_out(res, cfg)


# revision 29
# speedup vs baseline: 4.6500x; 1.0560x over previous
"""EvolveGCN (EGCN-H, 2 GRCU layers) Trainium2 Bass kernel, 8-way SPMD.

Strategy (dst-sharded graph parallel, transfer-optimized):
- 8 cores each own a contiguous range of N/8 destination nodes. Edges are
  routed (host-side) to their dst-owner core, grouped by 256-wide dst groups
  and by src half (int16 gather-index limit), padded to a fixed per-group
  subchunk count so the device program is static and identical on all cores.
- Host->device traffic is minimized (the axon tunnel is ~15-60MB/s and
  dominates wall time; device exec is ~0.15s):
  nodes ship SHARDED (N/8 rows per core) as 20-bit fixed-point planes
  (int16 hi + packed 4-bit nibbles, on-device: z=(hi+nib/16-0.5)/4096)
  and are AllGathered on-device over NeuronLink; gather indices ship as
  the 16-row wrapped band (replicated to 128 partitions on-device);
  dst-locations ship uint8; edge weights ship int16 fixed-point (w*32768,
  descaled for free via the PSUM->SBUF copy scale); h ships/stores fp16;
  the output ships int8 with per-(feature, 128-row-block) amax scales,
  dequantized host-side in assemble_out().
- Precision split (validated on the exact instance): layer-1 SpMM stays f32
  because the layer-2 top-k selection scores derive from h and the rank-128
  score gap is ~5e-3 - any fp16 rounding upstream of the scores flips the
  selected set and cascades through the sequential matrix GRU (rel err 0.3+).
  Layer-2's data path (h storage, gathers, S_T, both matmuls, output) is
  fp16/int8: selection happens before rounding, everything after is smooth.
  End-to-end sim (and measured device) rel err: 5.1e-3 vs the 2e-2 gate.
- segment_sum linearity: segsum(w * (Z@Q)[src], dst) == segsum(w*Z[src], dst) @ Q.
  Per 128-edge subchunk the core dma_gathers 128 rows of Z, builds the
  weighted one-hot S_T[e, d] = w_e * (dst_e == d) with one fused DVE
  tensor_scalar (is_equal x mult against a constant iota), and accumulates
  G.T = X.T @ S_T in PSUM with one matmul. After a group finishes:
  out = lrelu(G @ Q) via one more matmul.
- Layer boundary: per-step AllGather of the h slices (+ device-computed
  layer-2 scores); on-device exact top-k (vector.max8/max_index + global
  rank by count + indirect rank-scatter) and the 128x128 matrix GRU produce
  layer-2's evolved weights. Layer-1's evolved weights are host-precomputed
  (tiny sequential GRU on pure inputs, replicated - see sharding hint).
"""
import os
import sys

for _p in ("/opt/trn_rl_repo", "/root/.axon_site/_ro/trn_rl_repo"):
    if os.path.isdir(_p) and _p not in sys.path:
        sys.path.insert(0, _p)

import numpy as np

# Persistent jax compilation cache: run_bass_kernel_spmd re-jits a fresh
# closure per call, so without this every invocation re-runs the full
# walrus BIR->NEFF compile (~10-40s). The cache is keyed on the HLO hash;
# the executable still loads + runs on-device per call.
import jax as _jax

for _k, _v in (("jax_compilation_cache_dir", "/tmp/jax_cc_cache"),
               ("jax_persistent_cache_min_compile_time_secs", 0.0),
               ("jax_persistent_cache_min_entry_size_bytes", 0)):
    try:
        _jax.config.update(_k, _v)
    except Exception:
        pass

import concourse.bass as bass
import concourse.bacc as bacc
import concourse.mybir as mybir
import concourse.tile as tile
from concourse.bass_utils import run_bass_kernel_spmd

F32 = mybir.dt.float32
F16 = mybir.dt.float16
I16 = mybir.dt.int16
I32 = mybir.dt.int32
I8 = mybir.dt.int8
U8 = mybir.dt.uint8
ALU = mybir.AluOpType
ACT = mybir.ActivationFunctionType
SLOPE = float((1.0 / 8.0 + 1.0 / 3.0) / 2.0)  # rrelu eval-mode slope
WSCALE = 32768.0  # edge-weight int16 fixed-point scale
NSCALE = 4096.0   # nodes 20-bit fixed-point scale (range +-8, frac 2^-16)


class Cfg:
    def __init__(self, T, N, E, ncores, gw=256, topk_rounds=2):
        self.T, self.N, self.E, self.NCORES = T, N, E, ncores
        assert N % ncores == 0
        self.NPART = N // ncores          # dst nodes per core
        self.GW = gw                      # dst group width (matmul free dim)
        self.NG = -(-self.NPART // gw)    # groups per core
        self.SPLIT = min(32768 - (32768 % 128), -(-N // 2 // 128) * 128)
        if N <= 32767:
            self.SPLIT = -(-N // 2 // 128) * 128  # exercise both halves
        self.SPLIT = min(self.SPLIT, N)
        self.D = 128
        self.K = 128
        self.C_SC = -(-N // 128)          # score columns per partition
        self.PADN = 128 * self.C_SC
        self.R = topk_rounds              # rounds of per-partition max8
        self.NCAND = 128 * 8 * topk_rounds
        self.F_GH = None                  # set from data
        self.ncol = None
        self.ncol8 = None

    def set_fgh(self, f):
        self.F_GH = f
        self.ncol = self.NG * 2 * f           # metadata columns per t
        self.ncol8 = self.NG * 2 * f * 8      # idx columns per t


# ---------------------------------------------------------------- host prep
def _pack_edges(cfg, edge_src, edge_dst, edge_w):
    """Per-core static streams. Returns (idx, dstloc, w) arrays:
    idx   [NCORES, T, 16, ncol8]  int16   (16-row wrapped band)
    dstloc[NCORES, T, 128, ncol]  uint8   (values 0..GW-1)
    wpack [NCORES, T, 128, ncol]  int16   (w * 32768 fixed-point)
    """
    T, NG, GW, NPART, SPLIT = cfg.T, cfg.NG, cfg.GW, cfg.NPART, cfg.SPLIT
    NC = cfg.NCORES
    # fixed subchunks per (group, half): global max over all (core,t,g,half)
    maxc = 0
    percore_key = []
    for t in range(T):
        dst, src = edge_dst[t], edge_src[t]
        key = ((dst // NPART) * NG + (dst % NPART) // GW) * 2 + (src >= SPLIT)
        percore_key.append(key.astype(np.int64))
        maxc = max(maxc, int(np.bincount(key, minlength=NC * NG * 2).max()))
    F = -(-maxc // 128)
    cfg.set_fgh(F)

    idx = np.zeros((NC, T, 16, cfg.ncol8), np.int16)
    dstloc = np.zeros((NC, T, 128, cfg.ncol), np.uint8)
    wpack = np.zeros((NC, T, 128, cfg.ncol), np.int16)
    for t in range(T):
        dst, src, w = edge_dst[t], edge_src[t], edge_w[t]
        key = percore_key[t]
        order = np.argsort(key, kind="stable")
        key_s = key[order]
        src_s, dst_s, w_s = src[order], dst[order], w[order]
        cnt = np.bincount(key_s, minlength=NC * NG * 2)
        start = np.concatenate([[0], np.cumsum(cnt)[:-1]])
        pos = np.arange(len(key_s)) - start[key_s]  # position within block
        core = key_s // (NG * 2)
        blk = key_s % (NG * 2)                      # (g*2+half) within core
        i = pos                                     # stream slot in block
        s_sub, p_row = i // 128, i % 128
        col = blk * F + s_sub
        dl = (dst_s % NPART) % GW
        half = blk % 2
        iv = src_s - half * SPLIT
        assert iv.max() < 32768 and iv.min() >= 0
        dstloc[core, t, p_row, col] = dl.astype(np.uint8)
        wpack[core, t, p_row, col] = np.minimum(
            np.round(w_s.astype(np.float64) * WSCALE), WSCALE - 1).astype(np.int16)
        # 16-row wrapped indices
        r = i % 16
        j = blk * F * 8 + i // 16
        idx[core, t, r, j] = iv.astype(np.int16)
    return idx, dstloc, wpack


def _host_gru_layer1(cfg, nodes, W_init, scorer, gW, gU, gb):
    """Exact fp32 replica of the reference layer-1 weight evolution."""
    sn = np.float32(np.linalg.norm(scorer))
    Q = W_init.copy()
    qns = []
    for t in range(cfg.T):
        Z = nodes[t]
        scores = (Z @ scorer)[:, 0] / sn
        idx = np.argsort(-scores, kind="stable")[: cfg.K]
        z_topk = (Z[idx] * np.tanh(scores[idx])[:, None]).T
        upd = 1.0 / (1.0 + np.exp(-(gW[0] @ z_topk + gU[0] @ Q + gb[0])))
        rst = 1.0 / (1.0 + np.exp(-(gW[1] @ z_topk + gU[1] @ Q + gb[1])))
        h_cap = np.tanh(gW[2] @ z_topk + gU[2] @ (rst * Q) + gb[2])
        Q = (1.0 - upd) * Q + upd * h_cap
        qns.append(Q.copy())
    return np.stack(qns).astype(np.float32)


def _geom(cfg):
    """Group geometry: list of (r0, [(row0, width<=128), ...]) per group."""
    geom = []
    for g in range(cfg.NG):
        r0 = g * cfg.GW
        r1 = min(r0 + cfg.GW, cfg.NPART)
        hh = []
        x = r0
        while x < r1:
            wdt = min(128, r1 - x)
            hh.append((x, wdt))
            x += wdt
        geom.append((r0, hh))
    return geom


# ---------------------------------------------------------------- device build
def _build(cfg):
    nc = bacc.Bacc("TRN2", target_bir_lowering=False, debug=False,
                   num_devices=cfg.NCORES)
    T, N, D, GW, NG, F, NPART = cfg.T, cfg.N, cfg.D, cfg.GW, cfg.NG, cfg.F_GH, cfg.NPART
    SPLIT, C_SC, PADN, R = cfg.SPLIT, cfg.C_SC, cfg.PADN, cfg.R
    NCAND = cfg.NCAND
    core_ids = list(range(cfg.NCORES))

    def dram_in(name, shape, dtype=F32):
        return nc.dram_tensor(name, list(shape), dtype, kind="ExternalInput").ap()

    nhi_d = dram_in("nhi", (T, NPART, D), I16)        # round(z*4096)
    nlo_d = dram_in("nlo", (T, NPART, D // 2), U8)    # packed 4-bit refinement
    qn1 = dram_in("qn1", (T, D, D))
    gWT2 = dram_in("gWT2", (3, D, D))
    gUT2 = dram_in("gUT2", (3, D, D))
    gb2 = dram_in("gb2", (3, D, D))
    winit2 = dram_in("winit2", (D, D))
    scorer2 = dram_in("scorer2", (D, 1))          # pre-normalized
    iota_gw = dram_in("iota_gw", (128, GW))       # row = 0..GW-1, all partitions
    iota_col = dram_in("iota_col", (128, 1))      # p * C_SC
    identity = dram_in("identity", (128, 128))
    negpad = dram_in("negpad", (1, 128))          # -1e30 row
    idx_d = dram_in("idx", (T, 16, cfg.ncol8), I16)
    dstloc_d = dram_in("dstloc", (T, 128, cfg.ncol), U8)
    w_d = dram_in("wv", (T, 128, cfg.ncol), I16)

    geom = _geom(cfg)
    NBLK = sum(len(hh) for _, hh in geom)
    # int8 output, transposed [D, NPART] + per-(feature, row-block) amax scales
    out_d = nc.dram_tensor("out", [T, D, NPART], I8, kind="ExternalOutput").ap()
    scales_d = nc.dram_tensor("scales", [T, D, NBLK], F32,
                              kind="ExternalOutput").ap()

    with tile.TileContext(nc) as tc:
        import contextlib
        ctx = contextlib.ExitStack()
        with ctx:
            sb = ctx.enter_context(tc.tile_pool(name="sb", bufs=1))
            meta = ctx.enter_context(tc.tile_pool(name="meta", bufs=1))
            rcp = ctx.enter_context(tc.tile_pool(name="rcp", bufs=1))
            xgp = ctx.enter_context(tc.tile_pool(name="xgp", bufs=3))
            stp = ctx.enter_context(tc.tile_pool(name="stp", bufs=8))
            gtp = ctx.enter_context(tc.tile_pool(name="gtp", bufs=3))
            drp = ctx.enter_context(tc.tile_pool(name="drp", bufs=4))
            psg = ctx.enter_context(tc.tile_pool(name="psg", bufs=2, space="PSUM"))
            pso = ctx.enter_context(tc.tile_pool(name="pso", bufs=1, space="PSUM"))
            tkp = ctx.enter_context(tc.tile_pool(name="tkp", bufs=1))
            dram = ctx.enter_context(tc.tile_pool(name="dram", bufs=1, space="DRAM"))

            # constants
            iota_sb = sb.tile([128, GW], F32, tag="iota")
            nc.sync.dma_start(out=iota_sb[:], in_=iota_gw[:])
            ident_sb = sb.tile([128, 128], F32, tag="ident")
            nc.sync.dma_start(out=ident_sb[:], in_=identity[:])
            iotac_sb = sb.tile([128, 1], F32, tag="iotac")
            nc.sync.dma_start(out=iotac_sb[:], in_=iota_col[:])
            neg_sb = sb.tile([1, 128], F32, tag="negp")
            nc.sync.dma_start(out=neg_sb[:], in_=negpad[:])
            sc2_sb = sb.tile([128, 1], F32, tag="sc2")
            nc.sync.dma_start(out=sc2_sb[:], in_=scorer2[:])
            gW_sb, gU_sb, gb_sb = [], [], []
            for i in range(3):
                a = sb.tile([128, 128], F32, name=f"gw{i}", tag=f"gw{i}")
                nc.sync.dma_start(out=a[:], in_=gWT2[i])
                gW_sb.append(a)
                b = sb.tile([128, 128], F32, name=f"gu{i}", tag=f"gu{i}")
                nc.sync.dma_start(out=b[:], in_=gUT2[i])
                gU_sb.append(b)
                c = sb.tile([128, 128], F32, name=f"gb{i}", tag=f"gb{i}")
                nc.sync.dma_start(out=c[:], in_=gb2[i])
                gb_sb.append(c)
            qn1_sb = []
            for t in range(T):
                q = sb.tile([128, 128], F32, name=f"qn1_{t}", tag=f"qn1_{t}")
                nc.sync.dma_start(out=q[:], in_=qn1[t])
                qn1_sb.append(q)

            # persistent DRAM buffers
            nodes_sl = [dram.tile([NPART, D], F32, name=f"nsl{t}", tag=f"nsl{t}")
                        for t in range(T)]
            nodes_full = [dram.tile([N, D], F32, name=f"nfl{t}", tag=f"nfl{t}",
                                    addr_space="Shared") for t in range(T)]
            nodes_loc = [dram.tile([N, D], F32, name=f"nlc{t}", tag=f"nlc{t}")
                         for t in range(T)]
            h_slice = [dram.tile([NPART, D], F16, name=f"hsl{t}", tag=f"hsl{t}")
                       for t in range(T)]
            h_full = [dram.tile([N, D], F16, name=f"hfl{t}", tag=f"hfl{t}",
                                addr_space="Shared") for t in range(T)]
            sc_slice = [dram.tile([1, NPART], F32, name=f"ssl{t}", tag=f"ssl{t}")
                        for t in range(T)]
            h_loc = [dram.tile([N, D], F16, name=f"hlc{t}", tag=f"hlc{t}")
                     for t in range(T)]
            sc_full = [dram.tile([1, PADN], F32, name=f"sfl{t}", tag=f"sfl{t}",
                                 addr_space="Shared") for t in range(T)]
            cand_dram = dram.tile([1, NCAND], F32, tag="cand", bufs=2)
            sorted_dram = dram.tile([129, 2], F32, tag="sorted", bufs=2)

            qn2_sb = [sb.tile([128, 128], F32, name=f"qn2_{t}", tag=f"qn2_{t}")
                      for t in range(T)]
            qn2h_sb = [sb.tile([128, 128], F16, name=f"qn2h_{t}", tag=f"qn2h_{t}")
                       for t in range(T)]

            # reconstruct f32 nodes shard from 20-bit planes, then AllGather:
            # z = (hi + nib/16 - 0.5) / 4096, nibble pairs packed per byte
            FLAT = NPART * D // 128          # flat columns per t (partition-major)
            NCH = 5
            CH = FLAT // NCH
            CHN = CH // 2                    # packed-nibble columns per chunk
            assert CH * NCH == FLAT and CHN * 2 == CH
            for t in range(T):
                hi_flat = nhi_d[t].rearrange("a d -> (a d)").rearrange(
                    "(p c) -> p c", c=FLAT)
                lo_flat = nlo_d[t].rearrange("a d -> (a d)").rearrange(
                    "(p c) -> p c", c=FLAT // 2)
                sl_flat = nodes_sl[t][:].rearrange("a d -> (a d)").rearrange(
                    "(p c) -> p c", c=FLAT)
                for k in range(NCH):
                    cs = slice(k * CH, (k + 1) * CH)
                    csn = slice(k * CHN, (k + 1) * CHN)
                    rhi = rcp.tile([128, CH], I16, tag="rhi")
                    nc.sync.dma_start(out=rhi[:], in_=hi_flat[:, cs])
                    rhf = rcp.tile([128, CH], F32, tag="rhf")
                    nc.vector.tensor_copy(out=rhf[:], in_=rhi[:])
                    nb = rcp.tile([128, CHN], U8, tag="nb")
                    nc.sync.dma_start(out=nb[:], in_=lo_flat[:, csn])
                    nbf = rcp.tile([128, CHN], F32, tag="nbf")
                    nc.vector.tensor_copy(out=nbf[:], in_=nb[:])
                    # nibble split without mod (not a valid TS ISA op):
                    # t0 = b/16 - 0.5 + eps; a1 = round(t0) (int convert);
                    # even rt1 = a1/16 - 0.5; odd rt1 = t0 - a1 (carries a
                    # harmless +eps = 2^-6 bias, 3.8e-6 in z units)
                    t0 = rcp.tile([128, CHN], F32, tag="t0")
                    nc.vector.tensor_scalar(out=t0[:], in0=nbf[:],
                                            scalar1=float(1.0 / 16.0),
                                            scalar2=float(-0.5 + 1.0 / 64.0),
                                            op0=ALU.mult, op1=ALU.add)
                    ti = rcp.tile([128, CHN], I16, tag="ti")
                    nc.vector.tensor_copy(out=ti[:], in_=t0[:])
                    a1f = rcp.tile([128, CHN], F32, tag="a1f")
                    nc.vector.tensor_copy(out=a1f[:], in_=ti[:])
                    rt1 = rcp.tile([128, CH], F32, tag="rt1")
                    rt1v = rt1[:].rearrange("p (c two) -> p c two", two=2)
                    nc.vector.tensor_scalar(out=rt1v[:, :, 0], in0=a1f[:],
                                            scalar1=float(1.0 / 16.0),
                                            scalar2=-0.5,
                                            op0=ALU.mult, op1=ALU.add)
                    nc.vector.tensor_tensor(out=rt1v[:, :, 1], in0=t0[:],
                                            in1=a1f[:], op=ALU.subtract)
                    rt2 = rcp.tile([128, CH], F32, tag="rt2")
                    nc.vector.tensor_tensor(out=rt2[:], in0=rhf[:], in1=rt1[:],
                                            op=ALU.add)
                    rz = rcp.tile([128, CH], F32, tag="rz")
                    nc.vector.tensor_scalar(out=rz[:], in0=rt2[:],
                                            scalar1=float(1.0 / NSCALE),
                                            scalar2=None, op0=ALU.mult)
                    nc.sync.dma_start(out=sl_flat[:, cs], in_=rz[:])
                nc.gpsimd.collective_compute(
                    "AllGather", ALU.bypass,
                    replica_groups=[core_ids],
                    ins=[nodes_sl[t][:].opt()],
                    outs=[nodes_full[t][:].opt()])
                nc.sync.dma_start(out=nodes_loc[t][:], in_=nodes_full[t][:])

            def spmm_pass(t, z_src_ap, qn_tile, layer):
                """One (layer, t) SpMM pass. z_src_ap: [N, D] DRAM AP
                (f32 for layer 1, fp16 for layer 2)."""
                zdt = F32 if layer == 1 else F16
                idx_sb = meta.tile([128, cfg.ncol8], I16, tag="idx")
                for s in range(8):
                    nc.sync.dma_start(out=idx_sb[16 * s:16 * (s + 1), :],
                                      in_=idx_d[t])
                dl8_sb = meta.tile([128, cfg.ncol], U8, tag="dl8")
                nc.sync.dma_start(out=dl8_sb[:], in_=dstloc_d[t])
                dl_sb = meta.tile([128, cfg.ncol], F32, tag="dl")
                nc.vector.tensor_copy(out=dl_sb[:], in_=dl8_sb[:])
                wq_sb = meta.tile([128, cfg.ncol], I16, tag="wq")
                nc.sync.dma_start(out=wq_sb[:], in_=w_d[t])
                w_sb = meta.tile([128, cfg.ncol], F32, tag="wv")
                nc.vector.tensor_copy(out=w_sb[:], in_=wq_sb[:])
                z_lo = z_src_ap[0:SPLIT, :]
                z_hi = z_src_ap[SPLIT:N, :]
                if layer == 2:
                    sc8 = gtp.tile([128, NBLK], F32, tag="sc8")
                bi = 0
                for g in range(NG):
                    r0, hh = geom[g]
                    xg = []
                    for half, zsrc in ((0, z_lo), (1, z_hi)):
                        xt = xgp.tile([128, F * 128], zdt, tag="xg",
                                      name=f"xg{layer}_{t}_{g}_{half}")
                        c0 = (g * 2 + half) * F * 8
                        # single_packet SWDGE limit: <=64 desc/engine -> 1024 idxs
                        for s0 in range(0, F, 8):
                            ns = min(8, F - s0)
                            nc.gpsimd.dma_gather(
                                out_ap=xt[:, s0 * 128:(s0 + ns) * 128]
                                .rearrange("p (s e) -> p s e", e=128),
                                in_ap=zsrc,
                                idxs_ap=idx_sb[:, c0 + s0 * 8:c0 + (s0 + ns) * 8],
                                num_idxs=ns * 128,
                                num_idxs_reg=ns * 128,
                                elem_size=128,
                            )
                        xg.append(xt)
                    gt_ps = psg.tile([128, GW], F32, tag="gt", space="PSUM")
                    nmm = 2 * F
                    k = 0
                    for half in (0, 1):
                        for s in range(F):
                            col = (g * 2 + half) * F + s
                            st = stp.tile([128, GW], zdt, tag="st",
                                          name=f"st{layer}_{t}_{g}_{half}_{s}")
                            nc.vector.tensor_scalar(
                                out=st[:], in0=iota_sb[:],
                                scalar1=dl_sb[:, col:col + 1],
                                scalar2=w_sb[:, col:col + 1],
                                op0=ALU.is_equal, op1=ALU.mult)
                            lhs = xg[half][:, s * 128:(s + 1) * 128]
                            nc.tensor.matmul(out=gt_ps[:], lhsT=lhs, rhs=st[:],
                                             start=(k == 0), stop=(k == nmm - 1))
                            k += 1
                    # copy-out descales the int16 fixed-point edge weights
                    gt_sb = gtp.tile([128, GW], zdt, tag="gts")
                    nc.scalar.activation(out=gt_sb[:], in_=gt_ps[:], func=ACT.Copy,
                                         scale=float(1.0 / WSCALE))
                    for (rr, wdt) in hh:
                        o_ps = pso.tile([128, 128], F32, tag="ops", space="PSUM", bufs=2)
                        lhs2 = gt_sb[:, rr - r0:rr - r0 + wdt]
                        rhs2 = qn_tile[:]
                        nc.tensor.matmul(out=o_ps[:wdt, :], lhsT=lhs2, rhs=rhs2,
                                         start=True, stop=True)
                        sx = drp.tile([128, 128], F32, tag="sx")
                        nc.scalar.activation(out=sx[:wdt, :], in_=o_ps[:wdt, :],
                                             func=ACT.Copy, scale=SLOPE)
                        hb = drp.tile([128, 128], F32, tag="hb")
                        nc.vector.tensor_tensor(out=hb[:wdt, :], in0=o_ps[:wdt, :],
                                                in1=sx[:wdt, :], op=ALU.max)
                        # both layers transpose h (layer 1: scores; layer 2:
                        # per-feature int8 quantization on partitions)
                        ht_ps = pso.tile([128, 128], F32, tag="htp",
                                         space="PSUM")
                        nc.tensor.transpose(out=ht_ps[:, :wdt], in_=hb[:wdt, :],
                                            identity=ident_sb[:wdt, :wdt])
                        ht_sb = drp.tile([128, 128], F32, tag="hts")
                        nc.scalar.activation(out=ht_sb[:, :wdt],
                                             in_=ht_ps[:, :wdt], func=ACT.Copy)
                        if layer == 1:
                            hb16 = drp.tile([128, 128], F16, tag="hb16")
                            nc.vector.tensor_copy(out=hb16[:wdt, :],
                                                  in_=hb[:wdt, :])
                            nc.sync.dma_start(out=h_slice[t][rr:rr + wdt, :],
                                              in_=hb16[:wdt, :])
                            s_ps = pso.tile([1, 128], F32, tag="sps", space="PSUM")
                            nc.tensor.matmul(out=s_ps[:, :wdt], lhsT=sc2_sb[:],
                                             rhs=ht_sb[:, :wdt], start=True,
                                             stop=True)
                            s_sb = drp.tile([1, 128], F32, tag="ssb")
                            nc.scalar.activation(out=s_sb[:, :wdt],
                                                 in_=s_ps[:, :wdt], func=ACT.Copy)
                            nc.sync.dma_start(
                                out=sc_slice[t][:, rr:rr + wdt],
                                in_=s_sb[:1, :wdt])
                        else:
                            # int8 quantize per feature row of ht
                            mx = drp.tile([128, 1], F32, tag="qmx")
                            nc.vector.tensor_reduce(
                                out=mx[:], in_=ht_sb[:, :wdt],
                                axis=mybir.AxisListType.X, op=ALU.max)
                            mn = drp.tile([128, 1], F32, tag="qmn")
                            nc.vector.tensor_reduce(
                                out=mn[:], in_=ht_sb[:, :wdt],
                                axis=mybir.AxisListType.X, op=ALU.min)
                            nmn = drp.tile([128, 1], F32, tag="qnm")
                            nc.vector.tensor_scalar(out=nmn[:], in0=mn[:],
                                                    scalar1=-1.0, scalar2=None,
                                                    op0=ALU.mult)
                            am = drp.tile([128, 1], F32, tag="qam")
                            nc.vector.tensor_tensor(out=am[:], in0=mx[:],
                                                    in1=nmn[:], op=ALU.max)
                            amc = drp.tile([128, 1], F32, tag="qac")
                            nc.vector.tensor_scalar(out=amc[:], in0=am[:],
                                                    scalar1=1e-30, scalar2=None,
                                                    op0=ALU.max)
                            rc = drp.tile([128, 1], F32, tag="qrc")
                            nc.vector.reciprocal(out=rc[:], in_=amc[:])
                            inv = drp.tile([128, 1], F32, tag="qin")
                            nc.vector.tensor_scalar(out=inv[:], in0=rc[:],
                                                    scalar1=127.0, scalar2=None,
                                                    op0=ALU.mult)
                            q8 = drp.tile([128, 128], I8, tag="q8")
                            nc.vector.tensor_scalar(out=q8[:, :wdt],
                                                    in0=ht_sb[:, :wdt],
                                                    scalar1=inv[:, 0:1],
                                                    scalar2=None, op0=ALU.mult)
                            nc.vector.tensor_copy(out=sc8[:, bi:bi + 1],
                                                  in_=amc[:])
                            nc.sync.dma_start(out=out_d[t, :, rr:rr + wdt],
                                              in_=q8[:, :wdt])
                        bi += 1
                if layer == 2:
                    nc.sync.dma_start(out=scales_d[t], in_=sc8[:])
                if layer == 1:
                    nc.gpsimd.collective_compute(
                        "AllGather", ALU.bypass,
                        replica_groups=[core_ids],
                        ins=[h_slice[t][:].opt()],
                        outs=[h_full[t][:].opt()])
                    nc.sync.dma_start(out=h_loc[t][:], in_=h_full[t][:])
                    nc.gpsimd.collective_compute(
                        "AllGather", ALU.bypass,
                        replica_groups=[core_ids],
                        ins=[sc_slice[t][:].opt()],
                        outs=[sc_full[t][:, 0:N].opt()])

            def topk_gru(t, q_prev):
                """Exact top-128 of sc_full[t] + matrix GRU -> qn2_sb[t]."""
                S = tkp.tile([128, C_SC], F32, tag="S")
                nc.sync.dma_start(out=S[:],
                                  in_=sc_full[t][:].rearrange("o (p c) -> (o p) c",
                                                              c=C_SC))
                if PADN > N:
                    p_t, c_t = N // C_SC, N % C_SC
                    nc.sync.dma_start(out=S[p_t:p_t + 1, c_t:C_SC],
                                      in_=negpad[0:1, 0:C_SC - c_t])
                    if p_t + 1 < 128:
                        nc.sync.dma_start(
                            out=S[p_t + 1:128, :],
                            in_=negpad[0:1, 0:1].to_broadcast(
                                [127 - p_t, C_SC]))
                vals = tkp.tile([128, 8 * R], F32, tag="vals")
                cols = tkp.tile([128, 8 * R], F32, tag="cols")
                Swork = S
                for r in range(R):
                    mx = tkp.tile([128, 8], F32, tag="mx")
                    nc.vector.max(out=mx[:], in_=Swork[:])
                    ix = tkp.tile([128, 8], mybir.dt.uint32, tag="ix")
                    nc.vector.max_index(out=ix[:], in_max=mx[:], in_values=Swork[:])
                    nc.vector.tensor_copy(out=vals[:, r * 8:(r + 1) * 8], in_=mx[:])
                    nc.vector.tensor_copy(out=cols[:, r * 8:(r + 1) * 8], in_=ix[:])
                    if r < R - 1:
                        S2 = tkp.tile([128, C_SC], F32, tag=f"Sw{r % 2}")
                        nc.vector.match_replace(out=S2[:], in_to_replace=mx[:],
                                                in_values=Swork[:],
                                                imm_value=-1e30)
                        Swork = S2
                # global node id n = p*C_SC + col
                nid = tkp.tile([128, 8 * R], F32, tag="nid")
                nc.vector.tensor_scalar(out=nid[:], in0=cols[:],
                                        scalar1=iotac_sb[:, :1], scalar2=None,
                                        op0=ALU.add)
                # broadcast all candidates to all partitions via DRAM bounce
                nc.sync.dma_start(out=cand_dram[:], in_=vals[:])
                cb = tkp.tile([128, NCAND], F32, tag="cb")
                nc.sync.dma_start(out=cb[:],
                                  in_=cand_dram[:].to_broadcast([128, NCAND]))
                rank = tkp.tile([128, 8 * R], F32, tag="rank")
                for j in range(8 * R):
                    cmp = tkp.tile([128, NCAND], F32, tag="cmp")
                    nc.vector.tensor_scalar(out=cmp[:], in0=cb[:],
                                            scalar1=vals[:, j:j + 1], scalar2=None,
                                            op0=ALU.is_gt)
                    nc.vector.tensor_reduce(out=rank[:, j:j + 1], in_=cmp[:],
                                            axis=mybir.AxisListType.X, op=ALU.add)
                nc.vector.tensor_scalar(out=rank[:], in0=rank[:], scalar1=128.0,
                                        scalar2=None, op0=ALU.min)
                ri = tkp.tile([128, 8 * R], I32, tag="ri")
                nc.vector.tensor_copy(out=ri[:], in_=rank[:])
                pairs = tkp.tile([128, 16 * R], F32, tag="pairs")
                nc.vector.tensor_copy(
                    out=pairs[:].rearrange("p (j two) -> p j two", two=2)[:, :, 0],
                    in_=nid[:])
                nc.vector.tensor_copy(
                    out=pairs[:].rearrange("p (j two) -> p j two", two=2)[:, :, 1],
                    in_=vals[:])
                for j in range(8 * R):
                    nc.gpsimd.indirect_dma_start(
                        out=sorted_dram[:],
                        out_offset=bass.IndirectOffsetOnAxis(
                            ap=ri[:, j:j + 1], axis=0),
                        in_=pairs[:, 2 * j:2 * j + 2],
                        in_offset=None)
                sv = tkp.tile([128, 2], F32, tag="sv")
                nc.sync.dma_start(out=sv[:], in_=sorted_dram[0:128, :])
                nidx = tkp.tile([128, 1], I32, tag="nidx")
                nc.vector.tensor_copy(out=nidx[:], in_=sv[:, 0:1])
                tanhv = tkp.tile([128, 1], F32, tag="tanhv")
                nc.scalar.activation(out=tanhv[:], in_=sv[:, 1:2], func=ACT.Tanh)
                zsel16 = tkp.tile([128, 128], F16, tag="zsel16")
                nc.gpsimd.indirect_dma_start(
                    out=zsel16[:], out_offset=None,
                    in_=h_full[t][:],
                    in_offset=bass.IndirectOffsetOnAxis(ap=nidx[:, :1], axis=0))
                zsel = tkp.tile([128, 128], F32, tag="zsel")
                nc.vector.tensor_copy(out=zsel[:], in_=zsel16[:])
                zs2 = tkp.tile([128, 128], F32, tag="zs2")
                nc.scalar.activation(out=zs2[:], in_=zsel[:], func=ACT.Copy,
                                     scale=tanhv[:, :1])
                zt_ps = pso.tile([128, 128], F32, tag="ztp", space="PSUM")
                nc.tensor.transpose(out=zt_ps[:], in_=zs2[:], identity=ident_sb[:])
                ztop = tkp.tile([128, 128], F32, tag="ztop")
                nc.scalar.activation(out=ztop[:], in_=zt_ps[:], func=ACT.Copy)
                # matrix GRU
                gates = []
                rstq = None
                for i in range(3):
                    g_ps = pso.tile([128, 128], F32, tag="gps", space="PSUM")
                    nc.tensor.matmul(out=g_ps[:], lhsT=gW_sb[i][:], rhs=ztop[:],
                                     start=True, stop=False)
                    other = q_prev if i < 2 else rstq
                    nc.tensor.matmul(out=g_ps[:], lhsT=gU_sb[i][:], rhs=other[:],
                                     start=False, stop=True)
                    gsum = tkp.tile([128, 128], F32, tag=f"gsum{i}")
                    nc.vector.tensor_tensor(out=gsum[:], in0=g_ps[:],
                                            in1=gb_sb[i][:], op=ALU.add)
                    gact = tkp.tile([128, 128], F32, tag=f"gact{i}")
                    nc.scalar.activation(out=gact[:], in_=gsum[:],
                                         func=(ACT.Sigmoid if i < 2 else ACT.Tanh))
                    gates.append(gact)
                    if i == 1:
                        rstq = tkp.tile([128, 128], F32, tag="rstq")
                        nc.vector.tensor_tensor(out=rstq[:], in0=gates[1][:],
                                                in1=q_prev[:], op=ALU.mult)
                upd, h_cap = gates[0], gates[2]
                dql = tkp.tile([128, 128], F32, tag="dql")
                nc.vector.tensor_tensor(out=dql[:], in0=h_cap[:], in1=q_prev[:],
                                        op=ALU.subtract)
                udl = tkp.tile([128, 128], F32, tag="udl")
                nc.vector.tensor_tensor(out=udl[:], in0=upd[:], in1=dql[:],
                                        op=ALU.mult)
                nc.vector.tensor_tensor(out=qn2_sb[t][:], in0=q_prev[:],
                                        in1=udl[:], op=ALU.add)
                nc.vector.tensor_copy(out=qn2h_sb[t][:], in_=qn2_sb[t][:])
                return qn2_sb[t]

            # ---- program ----
            bisect = os.environ.get("KBISECT", "")
            if bisect.startswith("spmm"):
                npass = int(bisect[4:] or 2 * T)
                for i in range(npass):
                    spmm_pass(i % T, nodes_loc[i % T][:], qn1_sb[i % T], layer=1)
            else:
                for t in range(T):
                    spmm_pass(t, nodes_loc[t][:], qn1_sb[t], layer=1)
                qprev = sb.tile([128, 128], F32, name="winit2_sb", tag="winit2")
                nc.sync.dma_start(out=qprev[:], in_=winit2[:])
                for t in range(T):
                    qprev = topk_gru(t, qprev)
                for t in range(T):
                    spmm_pass(t, h_loc[t][:], qn2h_sb[t], layer=2)

    nc.compile()
    # memoize the BIR serialization: the module is immutable after compile,
    # but run_bass_via_pjrt re-lowers (and re-serializes ~77MB of BIR JSON)
    # on every invocation
    _tjb_cache = {}
    _orig_tjb = nc.to_json_bytes

    def _cached_tjb():
        if "b" not in _tjb_cache:
            _tjb_cache["b"] = _orig_tjb()
        return _tjb_cache["b"]

    nc.to_json_bytes = _cached_tjb
    return nc


# ---------------------------------------------------------------- entry point
_CACHE = {}
_LAST_IN_MAPS = None
_LAST_CFG = None

# full-problem constants (hardcoded per contract)
_T, _N, _E, _NCORES = 6, 50000, 1600000, 8


def _pack_nodes_20bit(zs):
    """[T,P,D] f32 -> (int16 hi, uint8 packed nibbles):
    z ~ (hi + nib/16 - 0.5)/4096, byte = nib[2j]*16 + nib[2j+1]."""
    s = zs.astype(np.float64) * NSCALE
    hi = np.round(s)
    assert np.abs(hi).max() < 32767, "nodes exceed 20-bit fixed-point range"
    nib = np.clip(np.round((s - hi + 0.5) * 16.0), 0, 15)
    b = nib.reshape(*nib.shape[:-1], nib.shape[-1] // 2, 2)
    lo = (b[..., 0] * 16 + b[..., 1]).astype(np.uint8)
    return hi.astype(np.int16), lo


def assemble_out(res, cfg=None):
    """Dequantize per-core int8 outputs -> full [T, N, D] f32."""
    cfg = cfg or _LAST_CFG
    geom = _geom(cfg)
    bi_of_row = np.zeros(cfg.NPART, np.int64)
    bi = 0
    for _, hh in geom:
        for (rr, wdt) in hh:
            bi_of_row[rr:rr + wdt] = bi
            bi += 1
    outs = []
    for c in range(cfg.NCORES):
        q = res.results[c]["out"].astype(np.float32)    # [T, D, NPART]
        am = res.results[c]["scales"]                   # [T, D, NBLK]
        amr = am[:, :, bi_of_row]                       # [T, D, NPART]
        outs.append(np.transpose(q * (amr * (1.0 / 127.0)), (0, 2, 1)))
    return np.concatenate(outs, axis=1).astype(np.float32)


def kernel(nodes, edge_src, edge_dst, edge_weight,
           W_init1, scorer1, gate_W1, gate_U1, gate_b1,
           W_init2, scorer2, gate_W2, gate_U2, gate_b2):
    nodes = np.ascontiguousarray(np.asarray(nodes, np.float32))
    T, N, D = nodes.shape
    E = np.asarray(edge_src).shape[1]
    gw = int(os.environ.get("KGW", "256"))
    cfg = Cfg(T, N, E, _NCORES, gw=gw, topk_rounds=2)
    idx, dstloc, wpack = _pack_edges(
        cfg, np.asarray(edge_src), np.asarray(edge_dst),
        np.asarray(edge_weight, np.float32))
    qn1 = _host_gru_layer1(cfg, nodes, np.asarray(W_init1, np.float32),
                           np.asarray(scorer1, np.float32),
                           np.asarray(gate_W1, np.float32),
                           np.asarray(gate_U1, np.float32),
                           np.asarray(gate_b1, np.float32))
    key = (T, N, E, cfg.F_GH, cfg.GW, cfg.R)
    if key not in _CACHE:
        _CACHE[key] = _build(cfg)
    nc = _CACHE[key]

    sc2n = (np.asarray(scorer2, np.float32)
            / np.float32(np.linalg.norm(scorer2))).astype(np.float32)
    shared = {
        "qn1": qn1,
        "gWT2": np.ascontiguousarray(
            np.transpose(np.asarray(gate_W2, np.float32), (0, 2, 1))),
        "gUT2": np.ascontiguousarray(
            np.transpose(np.asarray(gate_U2, np.float32), (0, 2, 1))),
        "gb2": np.asarray(gate_b2, np.float32),
        "winit2": np.asarray(W_init2, np.float32),
        "scorer2": sc2n,
        "iota_gw": np.tile(np.arange(cfg.GW, dtype=np.float32), (128, 1)),
        "iota_col": (np.arange(128, dtype=np.float32) * cfg.C_SC)[:, None],
        "identity": np.eye(128, dtype=np.float32),
        "negpad": np.full((1, 128), -1e30, np.float32),
    }
    in_maps = []
    for c in range(cfg.NCORES):
        m = dict(shared)
        nhi, nlo = _pack_nodes_20bit(
            nodes[:, c * cfg.NPART:(c + 1) * cfg.NPART, :])
        m["nhi"] = nhi
        m["nlo"] = nlo
        m["idx"] = idx[c]
        m["dstloc"] = dstloc[c]
        m["wv"] = wpack[c]
        in_maps.append(m)
    global _LAST_IN_MAPS, _LAST_CFG
    _LAST_IN_MAPS = in_maps
    _LAST_CFG = cfg
    res = run_bass_kernel_spmd(nc, in_maps, list(range(cfg.NCORES)))
    return assemble_out(res, cfg)
